# revision 1
# baseline (speedup 1.0000x reference)
"""GATv2 x2 + global-mean-pool + MLP head on 8 NeuronCores (Bass/Tile).

Sharding: destination-partitioned. Core c owns nodes [c*NPC, (c+1)*NPC);
it processes every edge whose dst is in its range, so attention softmax
segments are core-local (no cross-core softmax reductions).  Layer-1
node transforms are computed replicated; layer-2 source features are
AllGathered; mean-pool partials are AllReduced; the tiny dense head is
replicated.

|att| is folded into Wl/Wr/We on the host (channels permuted so
positive-att channels come first), so the per-edge attention logit is
    e = sum_c sign_c * leaky(t_c),  t = |att| * (xl[src]+xr[dst]+ew)
computed as two Prelu passes (the negative half uses scale=-0.2,
alpha=5, whose output is exactly -leaky(t)) + a free-dim reduce + exp.
1/|att| is folded into the next layer's weights (exact, host-side).
exp is applied without max-subtraction: logits are O(1) here, so this
is numerically identical to the reference softmax.
"""

import sys
import numpy as np
DEBUG = False
import ml_dtypes

sys.path.insert(0, "/opt/trn_rl_repo")

BF16 = ml_dtypes.bfloat16

DEFAULT_CFG = dict(
    N=50000, E=500000, G=64,
    DIN=128, ED=32, H1=256, H2=128, HD=64, OUT=8,
    NC=8, HALF=32768,
)


def _roundup(x, m):
    return (x + m - 1) // m * m


def _wrap16(idx, L):
    out = np.zeros((128, max(L // 16, 1)), np.int16)
    n = len(idx)
    if n:
        pos = np.arange(n)
        out[pos % 16, pos // 16] = idx.astype(np.int16)
    for g in range(1, 8):
        out[g * 16:(g + 1) * 16] = out[0:16]
    return out


def host_prep(inputs, cfg):
    c = dict(cfg)
    N, E, G = c["N"], c["E"], c["G"]
    DIN, ED, H1, H2 = c["DIN"], c["ED"], c["H1"], c["H2"]
    NCORE, HALF = c["NC"], c["HALF"]
    NPC = N // NCORE
    NBK = _roundup(NPC, 128) // 128
    BPC = NBK * 128
    NPAD1 = _roundup(N, 512)
    NPAD2 = NCORE * BPC

    f64 = lambda x: np.asarray(x, np.float64)
    att1, att2 = f64(inputs["att1"]), f64(inputs["att2"])
    a1 = np.maximum(np.abs(att1), 1e-12); s1 = np.where(att1 >= 0, 1.0, -1.0)
    a2 = np.maximum(np.abs(att2), 1e-12); s2 = np.where(att2 >= 0, 1.0, -1.0)
    perm1 = np.argsort(-s1, kind="stable"); P1 = int((s1 > 0).sum())
    perm2 = np.argsort(-s2, kind="stable"); P2 = int((s2 > 0).sum())
    a1p, a2p = a1[perm1], a2[perm2]

    Wl1p = (f64(inputs["Wl1"]) * a1)[:, perm1]
    Wr1p = (f64(inputs["Wr1"]) * a1)[:, perm1]
    We1p = (f64(inputs["We1"]) * a1)[:, perm1]
    bl1p = (f64(inputs["bl1"]) * a1)[perm1]
    br1p = (f64(inputs["br1"]) * a1)[perm1]
    b1p = (f64(inputs["b1"]) * a1)[perm1]

    Wl2u = f64(inputs["Wl2"])[perm1, :] / a1p[:, None]
    Wr2u = f64(inputs["Wr2"])[perm1, :] / a1p[:, None]
    Wl2pp = (Wl2u * a2)[:, perm2]
    Wr2pp = (Wr2u * a2)[:, perm2]
    We2p = (f64(inputs["We2"]) * a2)[:, perm2]
    bl2p = (f64(inputs["bl2"]) * a2)[perm2]
    br2p = (f64(inputs["br2"]) * a2)[perm2]
    b2p = (f64(inputs["b2"]) * a2)[perm2]

    Wd1u = f64(inputs["Wd1"])[perm2, :] / a2p[:, None]
    bs = f64(inputs["bn_gamma"]) / np.sqrt(f64(inputs["bn_var"]) + 1e-5)
    head_scale = bs
    head_bias = (f64(inputs["bd1"]) * bs + f64(inputs["bn_beta"])
                 - f64(inputs["bn_mean"]) * bs)

    src = np.asarray(inputs["edge_src"], np.int64)
    dst = np.asarray(inputs["edge_dst"], np.int64)
    batch = np.asarray(inputs["batch"], np.int64)
    eattr = np.asarray(inputs["edge_attr"], np.float64)

    core_of = dst // NPC
    blk_of = (dst % NPC) // 128
    dloc_of = (dst % NPC) % 128

    def layer_streams(row):
        half = (row >= HALF).astype(np.int64)
        cnt = np.zeros((NCORE, NBK, 2), np.int64)
        np.add.at(cnt, (core_of, blk_of, half), 1)
        seg = _roundup(cnt.max(axis=0), 128)           # [NBK, 2]
        seg[:, 0] = np.maximum(seg[:, 0], 128)
        offs = np.zeros((NBK, 2), np.int64)
        L = 0
        for b in range(NBK):
            for h in range(2):
                offs[b, h] = L
                L += seg[b, h]
        C = L // 128
        key = core_of * (NBK * 2) + blk_of * 2 + half
        order = np.argsort(key, kind="stable")
        ks = key[order]
        idxs = np.zeros((NCORE, 128, L // 16), np.int16)
        eT = np.zeros((NCORE, ED, L), BF16)
        eE = np.zeros((NCORE, 128, C, ED + 4), BF16)
        dstrow = np.full((NCORE, 1, L), 200.0, BF16)
        dloccol = np.full((NCORE, 128, C), 200.0, np.float32)
        bounds = np.searchsorted(ks, np.arange(NCORE * NBK * 2 + 1))
        for cr in range(NCORE):
            for b in range(NBK):
                for h in range(2):
                    k = cr * (NBK * 2) + b * 2 + h
                    m = order[bounds[k]:bounds[k + 1]]
                    n = len(m)
                    o = int(offs[b, h]); sl = int(seg[b, h])
                    if sl == 0:
                        continue
                    loc_idx = np.zeros(sl, np.int64)
                    loc_idx[:n] = row[m] - h * HALF
                    idxs[cr][:, o // 16:(o + sl) // 16] = _wrap16(loc_idx, sl)
                    if n:
                        eT[cr][:, o:o + n] = eattr[m].T.astype(BF16)
                        p = np.arange(n)
                        eE[cr][p % 128, o // 128 + p // 128, :ED] = eattr[m].astype(BF16)
                        eE[cr][p % 128, o // 128 + p // 128, ED] = BF16(1.0)
                        dstrow[cr][0, o:o + n] = dloc_of[m].astype(BF16)
                        dloccol[cr][p % 128, o // 128 + p // 128] = dloc_of[m]
        return dict(seg=seg, offs=offs, L=L, C=C, idxs=idxs, eT=eT, eE=eE,
                    dstrow=dstrow, dloccol=dloccol)

    row1 = src
    row2 = BPC * (src // NPC) + (src % NPC)
    L1s = layer_streams(row1)
    L2s = layer_streams(row2)

    cnts = np.maximum(np.bincount(batch, minlength=G).astype(np.float64), 1.0)
    PT = np.zeros((NCORE, NBK, 128, G), BF16)
    for cr in range(NCORE):
        for b in range(NBK):
            base = cr * NPC + b * 128
            nn = min(128, NPC - b * 128)
            if nn <= 0:
                continue
            gids = batch[base:base + nn]
            PT[cr, b, np.arange(nn), gids] = (1.0 / cnts[gids]).astype(BF16)

    iota_col = np.arange(128, dtype=np.float32).reshape(128, 1)
    IOTAF4 = np.tile(np.arange(128, dtype=np.float32)[None, :], (128, 4))
    IDENT = np.eye(128, dtype=BF16)
    IDENT32 = np.eye(128, dtype=np.float32)
    ones1 = np.ones((1, 128), BF16)
    ones_col = np.ones((128, 1), BF16)

    node_attr_T = np.zeros((DIN, NPAD1), BF16)
    node_attr_T[:, :N] = np.asarray(inputs["node_attr"], np.float32).T.astype(BF16)

    bcast = lambda v: np.tile(np.asarray(v, np.float32)[None, :], (128, 1)).copy()

    com = dict(
        node_attr_T=node_attr_T,
        Wl1p=Wl1p.astype(BF16), Wr1p=Wr1p.astype(BF16), We1p=We1p.astype(BF16),
        Wl2pp=Wl2pp.reshape(H1 // 128, 128, H2).transpose(1, 0, 2).reshape(128, -1).astype(BF16),
        Wr2pp=Wr2pp.reshape(H1 // 128, 128, H2).transpose(1, 0, 2).reshape(128, -1).astype(BF16),
        We2p=We2p.astype(BF16),
        bl1B=bcast(bl1p), br1B=bcast(br1p), b1B=bcast(b1p),
        bl2B=bcast(bl2p), br2B=bcast(br2p), b2B=bcast(b2p),
        a1p_col=a1p.astype(np.float32).reshape(H1 // 128, 128).T.copy(),
        Wd1u=Wd1u.astype(np.float32),
        head_scale=head_scale.astype(np.float32).reshape(-1, 1),
        head_bias=head_bias.astype(np.float32).reshape(-1, 1),
        Wd2=np.asarray(inputs["Wd2"], np.float32),
        bd2=np.asarray(inputs["bd2"], np.float32).reshape(-1, 1),
        iota_col=iota_col, IOTAF4=IOTAF4, IDENT=IDENT, IDENT32=IDENT32,
        ones1=ones1, ones_col=ones_col,
    )
    percore = []
    for cr in range(NCORE):
        percore.append(dict(
            idxs1=L1s["idxs"][cr], eT1=L1s["eT"][cr], eE1=L1s["eE"][cr],
            dstrow1=L1s["dstrow"][cr], dloccol1=L1s["dloccol"][cr],
            idxs2=L2s["idxs"][cr], eT2=L2s["eT"][cr], eE2=L2s["eE"][cr],
            dstrow2=L2s["dstrow"][cr], dloccol2=L2s["dloccol"][cr],
            PT=PT[cr],
        ))
    meta = dict(cfg=c, NPC=NPC, NBK=NBK, BPC=BPC, NPAD1=NPAD1, NPAD2=NPAD2,
                P1=P1, P2=P2, L1=L1s, L2=L2s)
    return com, percore, meta


def build_program(meta, com, pc0):
    import concourse.bass as bass
    import concourse.tile as tile
    from concourse import bacc, mybir
    from concourse import library_config

    c = meta["cfg"]
    G, H1, H2, OUT = c["G"], c["H1"], c["H2"], c["OUT"]
    NCORE = c["NC"]
    BPC = meta["BPC"]
    NPAD1, NPAD2 = meta["NPAD1"], meta["NPAD2"]
    dt = mybir.dt

    nc = bacc.Bacc("TRN2", target_bir_lowering=False, debug=False,
                   num_devices=NCORE)

    dmap = {np.dtype(np.float32): dt.float32, np.dtype(BF16): dt.bfloat16,
            np.dtype(np.int16): dt.int16}
    I = {}
    for d in (com, pc0):
        for k, a in d.items():
            I[k] = nc.dram_tensor(k, list(a.shape), dmap[a.dtype],
                                  kind="ExternalInput")

    out_t = nc.dram_tensor("out", [OUT, G], dt.float32, kind="ExternalOutput")
    tbl1 = nc.dram_tensor("tbl1", [NPAD1, H1], dt.bfloat16)
    dbg = dict(x1dbg=nc.dram_tensor("x1dbg", [meta["BPC"], H1], dt.float32),
               dendbg=nc.dram_tensor("dendbg", [meta["NBK"], 128], dt.float32))
    ag2_in = nc.dram_tensor("ag2_in", [BPC, H2], dt.bfloat16)
    tbl2 = nc.dram_tensor("tbl2", [NPAD2, H2], dt.bfloat16, addr_space="Shared")
    pool_in = nc.dram_tensor("pool_in", [G, H2], dt.float32)
    pool_out = nc.dram_tensor("pool_out", [G, H2], dt.float32, addr_space="Shared")

    with tile.TileContext(nc) as tc:
        _body(nc, tc, I, out_t, tbl1, ag2_in, tbl2, pool_in, pool_out,
              meta, bass, tile, mybir, library_config, dbg)
    nc.compile()
    return nc


def _body(nc, tc, I, out_t, tbl1, ag2_in, tbl2, pool_in, pool_out,
          meta, bass, tile, mybir, library_config, dbg=None):
    from contextlib import ExitStack

    c = meta["cfg"]
    G = c["G"]
    DIN, ED, H1, H2, HD, OUT = c["DIN"], c["ED"], c["H1"], c["H2"], c["HD"], c["OUT"]
    NCORE, HALF = c["NC"], c["HALF"]
    NPC, NBK, BPC = meta["NPC"], meta["NBK"], meta["BPC"]
    NPAD1, NPAD2 = meta["NPAD1"], meta["NPAD2"]
    P1, P2 = meta["P1"], meta["P2"]
    AF = mybir.ActivationFunctionType
    dt = mybir.dt
    Alu = mybir.AluOpType
    ds = bass.ds

    nc.gpsimd.load_library(library_config.mlp)
    pid = nc.partition_id()

    ctx = ExitStack()
    with ctx:
        consts = ctx.enter_context(tc.tile_pool(name="consts", bufs=1))

        def cload(name):
            a = I[name]
            t = consts.tile(list(a.shape), a.dtype, tag=name)
            nc.sync.dma_start(t[:], a[:])
            return t

        iota_col = cload("iota_col")
        IOTAF4 = cload("IOTAF4")
        IDENT = cload("IDENT")
        IDENT32 = cload("IDENT32")
        ones1 = cload("ones1")
        ones_col = cload("ones_col")
        Wl1p = cload("Wl1p"); Wr1p = cload("Wr1p"); We1p = cload("We1p")
        Wl2pp = cload("Wl2pp"); Wr2pp = cload("Wr2pp"); We2p = cload("We2p")
        bl1B = cload("bl1B"); br1B = cload("br1B"); b1B = cload("b1B")
        bl2B = cload("bl2B"); br2B = cload("br2B"); b2B = cload("b2B")
        a1p_col = cload("a1p_col")

        res = ctx.enter_context(tc.tile_pool(name="res", bufs=1))
        xr1_nm = res.tile([128, NBK, H1], dt.bfloat16, tag="xr1")
        x1_T = res.tile([128, H1 // 128, BPC], dt.bfloat16, tag="x1T")
        xr2_nm = res.tile([128, NBK, H2], dt.bfloat16, tag="xr2")

        # ---------------- phase 1: xl1 table (replicated) + xr1 (own) --
        with tc.tile_pool(name="p1sb", bufs=4) as p1sb, \
             tc.tile_pool(name="p1ps", bufs=3, space="PSUM") as p1ps:
            NT1 = NPAD1 // 512
            for t in range(NT1):
                rhs = p1sb.tile([DIN, 512], dt.bfloat16, tag="nat")
                nc.sync.dma_start(rhs[:], I["node_attr_T"][:, t * 512:(t + 1) * 512])
                for q in range(4):
                    ps = p1ps.tile([128, H1], dt.float32, tag="xlps")
                    nc.tensor.matmul(ps[:], rhs[:, q * 128:(q + 1) * 128], Wl1p[:],
                                     start=True, stop=True)
                    sb = p1sb.tile([128, H1], dt.bfloat16, tag="xlsb")
                    nc.vector.tensor_tensor(sb[:], ps[:], bl1B[:], op=Alu.add)
                    r0 = t * 512 + q * 128
                    nc.sync.dma_start(tbl1[r0:r0 + 128, :], sb[:])
            for b in range(NBK):
                rhs = p1sb.tile([DIN, 128], dt.bfloat16, tag="natr")
                nc.sync.dma_start(rhs[:], I["node_attr_T"][:, ds(pid * NPC + b * 128, 128)])
                ps = p1ps.tile([128, H1], dt.float32, tag="xlps")
                nc.tensor.matmul(ps[:], rhs[:], Wr1p[:], start=True, stop=True)
                nc.vector.tensor_tensor(xr1_nm[:, b, :], ps[:], br1B[:], op=Alu.add)

        # ---------------- shared edge phase ----------------------------
        def edge_phase(lay, pools, pool_ps=None, PT_sb=None):
            H = H1 if lay == 1 else H2
            Ppos = P1 if lay == 1 else P2
            We = We1p if lay == 1 else We2p
            xr_nm = xr1_nm if lay == 1 else xr2_nm
            bB = b1B if lay == 1 else b2B
            tbl = tbl1 if lay == 1 else tbl2
            rows = NPAD1 if lay == 1 else NPAD2
            sfx = str(lay)
            Ls = meta["L" + sfx]
            seg, offs = Ls["seg"], Ls["offs"]
            self_base = pid * (NPC if lay == 1 else BPC)
            sb, sbg, ps_s, ps_agg, ps_sm, ps_db = pools
            tlo = tbl[0:min(HALF, rows), :]
            thi = tbl[HALF:rows, :] if rows > HALF else None
            L = int(Ls["L"]); C = int(Ls["C"])
            pre = tc.alloc_tile_pool(name="pre" + sfx, bufs=1)
            idx_all = pre.tile([128, L // 16], dt.int16, tag="idxall")
            nc.sync.dma_start(idx_all[:], I["idxs" + sfx][:])
            eE_all = pre.tile([128, C, ED + 4], dt.bfloat16, tag="eEall")
            nc.scalar.dma_start(eE_all[:], I["eE" + sfx][:])
            dlc_all = pre.tile([128, C], dt.float32, tag="dlcall")
            nc.scalar.dma_start(dlc_all[:], I["dloccol" + sfx][:])

            for b in range(NBK):
                agg = ps_agg.tile([128, H + 4], dt.float32, tag="agg")
                laden = ps_sm.tile([128, ED + 4], dt.float32, tag="sm", name="laden")
                xlw = sbg.tile([128, H], dt.bfloat16, tag="xlw")
                nc.sync.dma_start(xlw[:], tbl[ds(self_base + b * 128, 128), :])
                first = True
                nreal = int(seg[b, 0] + seg[b, 1]) // 128
                cidx = 0
                for h in range(2):
                    sl = int(seg[b, h]); o = int(offs[b, h])
                    if sl == 0:
                        continue
                    xlg = sbg.tile([128, sl // 128, H], dt.bfloat16, tag="xlg")
                    nc.gpsimd.dma_gather(xlg[:], thi if h else tlo,
                                         idx_all[:, o // 16:(o + sl) // 16], sl, sl, H)
                    eTs = sb.tile([32, ((sl + 511) // 512) * 512], dt.bfloat16, tag="eT")
                    nc.sync.dma_start(eTs[:, :sl], I["eT" + sfx][:, o:o + sl])
                    drows = sb.tile([1, ((sl + 511) // 512) * 512], dt.bfloat16, tag="drow")
                    nc.sync.dma_start(drows[:, :sl], I["dstrow" + sfx][:, o:o + sl])
                    for po in range(0, sl, 512):
                        pl = min(512, sl - po)
                        nch = pl // 128
                        jj0 = (o + po) // 128
                        drow = drows[:, po:po + 512]
                        eTt = eTs[:, po:po + 512]
                        eEt = eE_all[:, jj0:jj0 + nch, :]
                        dlc = dlc_all[:, jj0:jj0 + nch]

                        dstB = ps_db.tile([128, 512], dt.float32, tag="dstB")
                        nc.tensor.matmul(dstB[:, :pl], ones1[:], drow[:, :pl],
                                         start=True, stop=True)
                        M = sb.tile([128, 512], dt.bfloat16, tag="M")
                        nc.vector.tensor_scalar(M[:, :pl], dstB[:, :pl], iota_col[:],
                                                None, op0=Alu.is_equal)
                        s4 = ps_s.tile([128, 4, H], dt.float32, tag="s4")
                        rpc = max(1, 2048 // (H * 4))  # chunks per psum zero-region
                        for j in range(nch):
                            cs = xlg[:, po // 128 + j, :]
                            nc.tensor.matmul(s4[:, j, :],
                                             eTt[:, j * 128:(j + 1) * 128], We[:],
                                             start=(j % rpc == 0), stop=False)
                            nc.tensor.matmul(s4[:, j, :],
                                             M[:, j * 128:(j + 1) * 128], xr_nm[:, b, :],
                                             start=False, stop=False)
                            nc.tensor.matmul(s4[:, j, :], IDENT[:], cs,
                                             start=False,
                                             stop=(j % rpc == rpc - 1 or j == nch - 1))
                        ls4 = sb.tile([128, 4, H], dt.bfloat16, tag="ls4")
                        if Ppos > 0:
                            nc.scalar.activation(ls4[:, :nch, 0:Ppos], s4[:, :nch, 0:Ppos],
                                                 AF.Prelu, alpha=0.2)
                        if Ppos < H:
                            nc.scalar.activation(ls4[:, :nch, Ppos:H], s4[:, :nch, Ppos:H],
                                                 AF.Prelu, scale=-0.2, alpha=5.0)
                        e4 = sb.tile([128, 4], dt.float32, tag="e4")
                        nc.vector.reduce_sum(e4[:, :nch], ls4[:, :nch, :],
                                             axis=mybir.AxisListType.X)
                        w4 = sb.tile([128, 4], dt.float32, tag="w4")
                        nc.scalar.activation(w4[:, :nch], e4[:, :nch], AF.Exp)
                        MT = sb.tile([128, 4, 128], dt.bfloat16, tag="MT")
                        nc.vector.tensor_tensor(
                            MT[:, :nch, :],
                            IOTAF4[:].rearrange("p (a b) -> p a b", b=128)[:, :nch, :],
                            dlc.to_broadcast((128, nch, 128)),
                            op=Alu.is_equal)
                        MwT = sb.tile([128, 4, 128], dt.bfloat16, tag="MwT")
                        nc.vector.tensor_tensor(
                            MwT[:, :nch, :], MT[:, :nch, :],
                            w4[:, :nch].to_broadcast((128, nch, 128)),
                            op=Alu.mult)
                        for j in range(nch):
                            cs = xlg[:, po // 128 + j, :]
                            nc.tensor.matmul(agg[:, 0:H], MwT[:, j, :], cs,
                                             start=first, stop=False)
                            nc.tensor.matmul(agg[:, H:H + 1], MwT[:, j, :], ones_col[:],
                                             start=False, stop=False)
                            nc.tensor.matmul(laden[:, 0:ED + 2], MT[:, j, :],
                                             eE_all[:, jj0 + j, 0:ED + 2],
                                             start=(cidx == 0), stop=(cidx == nreal - 1))
                            first = False
                            cidx += 1
                # loop_attr finalize
                deg = sb.tile([128, 1], dt.float32, tag="deg")
                nc.vector.tensor_scalar(deg[:], laden[:, ED:ED + 1], 1.0, None, op0=Alu.max)
                rdeg = sb.tile([128, 1], dt.float32, tag="rdeg")
                nc.vector.reciprocal(rdeg[:], deg[:])
                la_sb = sb.tile([128, ED], dt.bfloat16, tag="lasb")
                nc.vector.tensor_scalar(la_sb[:], laden[:, 0:ED], rdeg[:], None, op0=Alu.mult)
                laT_ps = ps_sm.tile([ED, 128], dt.bfloat16, tag="sm")
                nc.tensor.transpose(laT_ps[:], la_sb[:], IDENT[:])
                laT = sb.tile([ED, 128], dt.bfloat16, tag="laTsb")
                nc.scalar.copy(laT[:], laT_ps[:])
                # self chunk
                s_s = ps_s.tile([128, 4, H], dt.float32, tag="s4")
                nc.tensor.matmul(s_s[:, 0, :], laT[:], We[:], start=True, stop=False)
                nc.tensor.matmul(s_s[:, 0, :], IDENT[:], xr_nm[:, b, :], start=False, stop=False)
                nc.tensor.matmul(s_s[:, 0, :], IDENT[:], xlw[:], start=False, stop=True)
                ls_s = sb.tile([128, 4, H], dt.bfloat16, tag="ls4")
                if Ppos > 0:
                    nc.scalar.activation(ls_s[:, 0, 0:Ppos], s_s[:, 0, 0:Ppos],
                                         AF.Prelu, alpha=0.2)
                if Ppos < H:
                    nc.scalar.activation(ls_s[:, 0, Ppos:H], s_s[:, 0, Ppos:H],
                                         AF.Prelu, scale=-0.2, alpha=5.0)
                es = sb.tile([128, 1], dt.float32, tag="es")
                nc.vector.reduce_sum(es[:], ls_s[:, 0:1, :], axis=mybir.AxisListType.X)
                ws = sb.tile([128, 1], dt.float32, tag="ws")
                nc.scalar.activation(ws[:], es[:], AF.Exp)
                diagw = sb.tile([128, 128], dt.bfloat16, tag="diagw")
                nc.vector.tensor_scalar(diagw[:], IDENT[:], ws[:], None, op0=Alu.mult)
                nc.tensor.matmul(agg[:, 0:H], diagw[:], xlw[:], start=False, stop=False)
                nc.tensor.matmul(agg[:, H:H + 1], diagw[:], ones_col[:], start=False, stop=True)
                # finalize block: x = relu(agg/den + b)
                rden = sb.tile([128, 1], dt.float32, tag="rden")
                nc.vector.reciprocal(rden[:], agg[:, H:H + 1])
                t1 = sb.tile([128, H], dt.float32, tag="t1")
                nc.vector.tensor_scalar(t1[:], agg[:, 0:H], rden[:], None, op0=Alu.mult)
                t2 = sb.tile([128, H], dt.float32, tag="t2")
                nc.vector.tensor_tensor(t2[:], t1[:], bB[:], op=Alu.add)
                x_nm = sb.tile([128, H], dt.bfloat16, tag="xnm")
                nc.scalar.activation(x_nm[:], t2[:], AF.Relu)
                if lay == 1 and DEBUG:
                    nc.sync.dma_start(dbg["x1dbg"][b * 128:(b + 1) * 128, :], t2[:])
                    nc.sync.dma_start(dbg["dendbg"][b, :], rden[:, 0])
                if lay == 1:
                    for hh in range(H1 // 128):
                        tp = ps_sm.tile([128, 128], dt.bfloat16, tag="sm")
                        nc.tensor.transpose(tp[:], x_nm[:, hh * 128:(hh + 1) * 128], IDENT[:])
                        nc.scalar.copy(x1_T[:, hh, b * 128:(b + 1) * 128], tp[:])
                else:
                    nc.tensor.matmul(pool_ps[:, 0:H2], PT_sb[b][:], x_nm[:],
                                     start=(b == 0), stop=(b == NBK - 1))
            pre.release()

        # layer-1 edge phase
        with ExitStack() as ctx1:
            pools = (
                ctx1.enter_context(tc.tile_pool(name="sb1", bufs=4)),
                ctx1.enter_context(tc.tile_pool(name="sbg1", bufs=4)),
                ctx1.enter_context(tc.tile_pool(name="ps_s1", bufs=2, space="PSUM")),
                ctx1.enter_context(tc.tile_pool(name="ps_agg1", bufs=2, space="PSUM")),
                ctx1.enter_context(tc.tile_pool(name="ps_sm1", bufs=1, space="PSUM")),
                ctx1.enter_context(tc.tile_pool(name="ps_db1", bufs=1, space="PSUM")),
            )
            edge_phase(1, pools)

        # ---------------- layer-2 node transforms + AllGather ----------
        with tc.tile_pool(name="p2sb", bufs=4) as p2sb, \
             tc.tile_pool(name="p2ps", bufs=4, space="PSUM") as p2ps:
            for b in range(NBK):
                ps = p2ps.tile([128, H2], dt.float32, tag="xl2ps")
                for hh in range(H1 // 128):
                    nc.tensor.matmul(ps[:], x1_T[:, hh, b * 128:(b + 1) * 128],
                                     Wl2pp[:, hh * H2:(hh + 1) * H2],
                                     start=(hh == 0), stop=(hh == H1 // 128 - 1))
                sbx = p2sb.tile([128, H2], dt.bfloat16, tag="xl2sb")
                nc.vector.tensor_tensor(sbx[:], ps[:], bl2B[:], op=Alu.add)
                nc.sync.dma_start(ag2_in[b * 128:(b + 1) * 128, :], sbx[:])
                ps2 = p2ps.tile([128, H2], dt.float32, tag="xr2ps")
                for hh in range(H1 // 128):
                    nc.tensor.matmul(ps2[:], x1_T[:, hh, b * 128:(b + 1) * 128],
                                     Wr2pp[:, hh * H2:(hh + 1) * H2],
                                     start=(hh == 0), stop=(hh == H1 // 128 - 1))
                nc.vector.tensor_tensor(xr2_nm[:, b, :], ps2[:], br2B[:], op=Alu.add)
        nc.gpsimd.collective_compute(
            "AllGather", mybir.AluOpType.bypass,
            replica_groups=[list(range(NCORE))],
            ins=[ag2_in[:]], outs=[tbl2[:]])

        # ---------------- layer-2 edge phase + pooling ------------------
        pool_pp = ctx.enter_context(tc.tile_pool(name="poolps", bufs=1, space="PSUM"))
        pool_ps = pool_pp.tile([G, H2 + 4], dt.float32, tag="pool")
        pt_pool = ctx.enter_context(tc.tile_pool(name="ptsb", bufs=1))
        PT_sb = []
        for b in range(NBK):
            t = pt_pool.tile([128, G], dt.bfloat16, tag=f"pt{b}")
            nc.sync.dma_start(t[:], I["PT"][b])
            PT_sb.append(t)
        with ExitStack() as ctx2:
            pools = (
                ctx2.enter_context(tc.tile_pool(name="sb2", bufs=4)),
                ctx2.enter_context(tc.tile_pool(name="sbg2", bufs=4)),
                ctx2.enter_context(tc.tile_pool(name="ps_s2", bufs=2, space="PSUM")),
                ctx2.enter_context(tc.tile_pool(name="ps_agg2", bufs=2, space="PSUM")),
                ctx2.enter_context(tc.tile_pool(name="ps_sm2", bufs=1, space="PSUM")),
                ctx2.enter_context(tc.tile_pool(name="ps_db2", bufs=1, space="PSUM")),
            )
            edge_phase(2, pools, pool_ps=pool_ps, PT_sb=PT_sb)

        # ---------------- head -----------------------------------------
        with tc.tile_pool(name="hsb", bufs=2) as hsb, \
             tc.tile_pool(name="hps", bufs=2, space="PSUM") as hps:
            psb = hsb.tile([G, H2], dt.float32, tag="poolsb")
            nc.scalar.copy(psb[:], pool_ps[:, 0:H2])
            nc.sync.dma_start(pool_in[:], psb[:])
            nc.gpsimd.collective_compute(
                "AllReduce", mybir.AluOpType.add,
                replica_groups=[list(range(NCORE))],
                ins=[pool_in[:]], outs=[pool_out[:]])
            pooled = hsb.tile([G, H2], dt.float32, tag="pooled")
            nc.sync.dma_start(pooled[:], pool_out[:])
            pooled_T_ps = hps.tile([H2, G], dt.float32, tag="pooledT")
            nc.tensor.transpose(pooled_T_ps[:], pooled[:], IDENT32[0:G, 0:G])
            pooled_T = hsb.tile([H2, G], dt.float32, tag="pooledTsb")
            nc.scalar.copy(pooled_T[:], pooled_T_ps[:])
            Wd1sb = hsb.tile([H2, HD], dt.float32, tag="wd1")
            nc.sync.dma_start(Wd1sb[:], I["Wd1u"][:])
            h1ps = hps.tile([HD, G], dt.float32, tag="h1")
            nc.tensor.matmul(h1ps[:], Wd1sb[:], pooled_T[:], start=True, stop=True)
            hscale = hsb.tile([HD, 1], dt.float32, tag="hscale")
            nc.sync.dma_start(hscale[:], I["head_scale"][:])
            hbias = hsb.tile([HD, 1], dt.float32, tag="hbias")
            nc.sync.dma_start(hbias[:], I["head_bias"][:])
            th = hsb.tile([HD, G], dt.float32, tag="th")
            nc.scalar.activation(th[:], h1ps[:], AF.Prelu, bias=hbias[:],
                                 scale=hscale[:], alpha=0.1)
            Wd2sb = hsb.tile([HD, OUT], dt.float32, tag="wd2")
            nc.sync.dma_start(Wd2sb[:], I["Wd2"][:])
            ops = hps.tile([OUT, G], dt.float32, tag="ops")
            nc.tensor.matmul(ops[:], Wd2sb[:], th[:], start=True, stop=True)
            bd2sb = hsb.tile([OUT, 1], dt.float32, tag="bd2sb")
            nc.sync.dma_start(bd2sb[:], I["bd2"][:])
            osb = hsb.tile([OUT, G], dt.float32, tag="osb")
            nc.vector.tensor_scalar(osb[:], ops[:], bd2sb[:], None, op0=Alu.add)
            nc.sync.dma_start(out_t[:], osb[:])


def _kernel(inputs, cfg, runner=None, trace=False):
    com, percore, meta = host_prep(inputs, cfg)
    nc = build_program(meta, com, percore[0])
    in_maps = [dict(com, **pc) for pc in percore]
    if runner is None:
        from concourse.bass_utils import run_bass_kernel_spmd
        res = run_bass_kernel_spmd(nc, in_maps, list(range(cfg["NC"])), trace=trace)
        out = np.asarray(res.results[0]["out"])
        return out.T.copy().astype(np.float32), res
    return runner(nc, in_maps)


def kernel(**inputs):
    out, _ = _kernel(inputs, DEFAULT_CFG)
    return out



# revision 23
# speedup vs baseline: 1.0568x; 1.0568x over previous
"""GATv2 x2 + global-mean-pool + MLP head on 8 NeuronCores (Bass/Tile).

Sharding: destination-partitioned. Core c owns nodes [c*NPC, (c+1)*NPC);
it processes every edge whose dst is in its range, so attention softmax
segments are core-local.

Layer 1 gathers RAW node_attr rows (256B) per edge — no xl1 table is
ever materialized.  The per-chunk score xl-term is a matmul of the
transposed gathered rows with Wl1; the aggregation accumulates
U_T[DIN, d] += NA_g.T @ MwT per chunk and applies Wl1 once per block.
Layer 2 AllGathers the raw xl2 table (x1 @ Wl2, no bias) and gathers
its 256B rows per edge.

Host precomputes: loop_attr (self-loop edge features), per-chunk dst
one-hot matrices M [dstrow, edge] (streamed, fp8), dst-local-row
columns, and folds |att| into the weights (channels permuted so
positive-att channels come first; see baseline notes).  All biases are
folded: the score-side bias (bl+br) rides on xr; the output-side bias
(bl+b) is added at block finalize (valid since softmax weights sum to
1).  exp is applied without max-subtraction: logits are O(1) here.
"""

import sys
import numpy as np
import ml_dtypes

sys.path.insert(0, "/opt/trn_rl_repo")

BF16 = ml_dtypes.bfloat16
F8 = ml_dtypes.float8_e4m3

DEFAULT_CFG = dict(
    N=50000, E=500000, G=64,
    DIN=128, ED=32, H1=256, H2=128, HD=64, OUT=8,
    NC=8, HALF=32768,
)


def _roundup(x, m):
    return (x + m - 1) // m * m


def _wrap16(idx, L):
    out = np.full((128, max(L // 16, 1)), -1, np.int16)
    n = len(idx)
    if n:
        pos = np.arange(n)
        out[pos % 16, pos // 16] = idx.astype(np.int16)
    for g in range(1, 8):
        out[g * 16:(g + 1) * 16] = out[0:16]
    return out


def host_prep(inputs, cfg):
    c = dict(cfg)
    N, E, G = c["N"], c["E"], c["G"]
    DIN, ED, H1, H2 = c["DIN"], c["ED"], c["H1"], c["H2"]
    NCORE, HALF = c["NC"], c["HALF"]
    NPC = N // NCORE
    NBK = _roundup(NPC, 128) // 128
    BPC = NBK * 128
    NPAD1 = _roundup(N, 512)
    NPAD2 = NCORE * BPC

    f64 = lambda x: np.asarray(x, np.float64)
    att1, att2 = f64(inputs["att1"]), f64(inputs["att2"])
    a1 = np.maximum(np.abs(att1), 1e-12); s1 = np.where(att1 >= 0, 1.0, -1.0)
    a2 = np.maximum(np.abs(att2), 1e-12); s2 = np.where(att2 >= 0, 1.0, -1.0)
    perm1 = np.argsort(-s1, kind="stable"); P1 = int((s1 > 0).sum())
    perm2 = np.argsort(-s2, kind="stable"); P2 = int((s2 > 0).sum())
    a1p, a2p = a1[perm1], a2[perm2]

    Wl1p = (f64(inputs["Wl1"]) * a1)[:, perm1]
    Wr1p = (f64(inputs["Wr1"]) * a1)[:, perm1]
    We1p = (f64(inputs["We1"]) * a1)[:, perm1]
    bl1p = (f64(inputs["bl1"]) * a1)[perm1]
    br1p = (f64(inputs["br1"]) * a1)[perm1]
    b1p = (f64(inputs["b1"]) * a1)[perm1]

    Wl2u = f64(inputs["Wl2"])[perm1, :] / a1p[:, None]
    Wr2u = f64(inputs["Wr2"])[perm1, :] / a1p[:, None]
    Wl2pp = (Wl2u * a2)[:, perm2]
    Wr2pp = (Wr2u * a2)[:, perm2]
    We2p = (f64(inputs["We2"]) * a2)[:, perm2]
    bl2p = (f64(inputs["bl2"]) * a2)[perm2]
    br2p = (f64(inputs["br2"]) * a2)[perm2]
    b2p = (f64(inputs["b2"]) * a2)[perm2]

    Wd1u = f64(inputs["Wd1"])[perm2, :] / a2p[:, None]
    bs = f64(inputs["bn_gamma"]) / np.sqrt(f64(inputs["bn_var"]) + 1e-5)
    head_scale = bs
    head_bias = (f64(inputs["bd1"]) * bs + f64(inputs["bn_beta"])
                 - f64(inputs["bn_mean"]) * bs)

    src = np.asarray(inputs["edge_src"], np.int64)
    dst = np.asarray(inputs["edge_dst"], np.int64)
    batch = np.asarray(inputs["batch"], np.int64)
    eattr = np.asarray(inputs["edge_attr"], np.float64)

    # loop_attr (self-loop edge features) on host: segment mean of eattr by dst
    deg = np.bincount(dst, minlength=N).astype(np.float64)
    order_d = np.argsort(dst, kind="stable")
    eattr_sorted = eattr[order_d]
    cuts = np.searchsorted(dst[order_d], np.arange(N))
    la = np.zeros((N, ED), np.float64)
    nz = deg > 0
    sums = np.add.reduceat(eattr_sorted, np.minimum(cuts, len(dst) - 1), axis=0)
    la[nz] = sums[nz] / deg[nz][:, None]

    core_of = dst // NPC
    blk_of = (dst % NPC) // 128
    dloc_of = (dst % NPC) % 128

    def layer_streams(row):
        half = (row >= HALF).astype(np.int64)
        cnt = np.zeros((NCORE, NBK, 2), np.int64)
        np.add.at(cnt, (core_of, blk_of, half), 1)
        nmax = cnt.max(axis=0)                         # [NBK, 2] real rows
        nmax[:, 0] = np.maximum(nmax[:, 0], 1)
        seg = _roundup(nmax, 128)                      # [NBK, 2]
        seg[:, 0] = np.maximum(seg[:, 0], 128)
        offs = np.zeros((NBK, 2), np.int64)
        L = 0
        for b in range(NBK):
            for h in range(2):
                offs[b, h] = L
                L += seg[b, h]
        C = L // 128
        key = core_of * (NBK * 2) + blk_of * 2 + half
        order = np.argsort(key, kind="stable")
        ks = key[order]
        idxs = np.zeros((NCORE, 128, L // 16), np.int16)
        eT = np.zeros((NCORE, ED, L), BF16)
        M8 = np.zeros((NCORE, 128, L), F8)
        dlc = np.full((NCORE, 128, C), 200.0, np.float32)
        bounds = np.searchsorted(ks, np.arange(NCORE * NBK * 2 + 1))
        for cr in range(NCORE):
            for b in range(NBK):
                for h in range(2):
                    k = cr * (NBK * 2) + b * 2 + h
                    m = order[bounds[k]:bounds[k + 1]]
                    n = len(m)
                    o = int(offs[b, h]); sl = int(seg[b, h])
                    if sl == 0:
                        continue
                    nm = int(nmax[b, h])
                    loc_idx = np.full(sl, -1, np.int64)
                    loc_idx[:nm] = 0
                    loc_idx[:n] = row[m] - h * HALF
                    idxs[cr][:, o // 16:(o + sl) // 16] = _wrap16(loc_idx, sl)
                    if n:
                        eT[cr][:, o:o + n] = eattr[m].T.astype(BF16)
                        M8[cr][dloc_of[m], o + np.arange(n)] = F8(1.0)
                        p = np.arange(n)
                        dlc[cr][p % 128, o // 128 + p // 128] = dloc_of[m]
        return dict(seg=seg, offs=offs, nmax=nmax, L=L, C=C, idxs=idxs, eT=eT,
                    M8=M8, dlc=dlc)

    row1 = src
    row2 = BPC * (src // NPC) + (src % NPC)
    L1s = layer_streams(row1)
    L2s = layer_streams(row2)
    CHMX = int(max(L1s["seg"].max(), L2s["seg"].max())) // 128

    cnts = np.maximum(np.bincount(batch, minlength=G).astype(np.float64), 1.0)
    PT = np.zeros((NCORE, NBK, 128, G), BF16)
    for cr in range(NCORE):
        for b in range(NBK):
            base = cr * NPC + b * 128
            nn = min(128, NPC - b * 128)
            if nn <= 0:
                continue
            gids = batch[base:base + nn]
            PT[cr, b, np.arange(nn), gids] = (1.0 / cnts[gids]).astype(BF16)

    iota_col = np.arange(128, dtype=np.float32).reshape(128, 1)
    IOTAF4 = np.tile(np.arange(128, dtype=np.float32)[None, :], (128, 4))
    IDENT = np.eye(128, dtype=BF16)
    IDENT32 = np.eye(128, dtype=np.float32)
    ones_col = np.ones((128, 1), BF16)

    na_bf = np.zeros((NPAD1, DIN), BF16)
    na_f32 = np.asarray(inputs["node_attr"], np.float32)
    na_bf[:N] = na_f32.astype(BF16)

    # per-core own-node views (self-loop chunks + xr transform)
    natT = np.zeros((NCORE, DIN, NBK, 128), BF16)
    na_own = np.zeros((NCORE, 128, NBK, DIN), BF16)
    laT = np.zeros((NCORE, ED, NBK, 128), BF16)
    for cr in range(NCORE):
        for b in range(NBK):
            base = cr * NPC + b * 128
            nn = min(128, N - base) if base < N else 0
            nn = min(nn, NPC - b * 128)
            if nn <= 0:
                continue
            natT[cr, :, b, :nn] = na_f32[base:base + nn].T.astype(BF16)
            na_own[cr, :nn, b, :] = na_f32[base:base + nn].astype(BF16)
            laT[cr, :, b, :nn] = la[base:base + nn].T.astype(BF16)

    bcast = lambda v: np.tile(np.asarray(v, np.float32)[None, :], (128, 1)).copy()

    com = dict(
        na_bf=na_bf,
        Wl1p=Wl1p.astype(BF16), Wr1p=Wr1p.astype(BF16), We1p=We1p.astype(BF16),
        Wl2pp=Wl2pp.reshape(H1 // 128, 128, H2).transpose(1, 0, 2).reshape(128, -1).astype(BF16),
        Wr2pp=Wr2pp.reshape(H1 // 128, 128, H2).transpose(1, 0, 2).reshape(128, -1).astype(BF16),
        We2p=We2p.astype(BF16),
        brB1=bcast(bl1p + br1p), bB1=bcast(bl1p + b1p),
        brB2=bcast(bl2p + br2p), bB2=bcast(bl2p + b2p),
        Wd1u=Wd1u.astype(np.float32),
        head_scale=head_scale.astype(np.float32).reshape(-1, 1),
        head_bias=head_bias.astype(np.float32).reshape(-1, 1),
        Wd2=np.asarray(inputs["Wd2"], np.float32),
        bd2=np.asarray(inputs["bd2"], np.float32).reshape(-1, 1),
        iota_col=iota_col, IOTAF4=IOTAF4, IDENT=IDENT, IDENT32=IDENT32,
        ones_col=ones_col,
    )
    percore = []
    for cr in range(NCORE):
        percore.append(dict(
            idxs1=L1s["idxs"][cr], eT1=L1s["eT"][cr], M81=L1s["M8"][cr],
            dlc1=L1s["dlc"][cr],
            idxs2=L2s["idxs"][cr], eT2=L2s["eT"][cr], M82=L2s["M8"][cr],
            dlc2=L2s["dlc"][cr],
            PT=PT[cr], natT=natT[cr], na_own=na_own[cr], laT=laT[cr],
        ))
    meta = dict(cfg=c, NPC=NPC, NBK=NBK, BPC=BPC, NPAD1=NPAD1, NPAD2=NPAD2,
                P1=P1, P2=P2, L1=L1s, L2=L2s, CHMX=CHMX)
    return com, percore, meta


def build_program(meta, com, pc0):
    import concourse.bass as bass
    import concourse.tile as tile
    from concourse import bacc, mybir
    from concourse import library_config

    c = meta["cfg"]
    G, H2, OUT = c["G"], c["H2"], c["OUT"]
    NCORE = c["NC"]
    BPC = meta["BPC"]
    NPAD2 = meta["NPAD2"]
    dt = mybir.dt

    nc = bacc.Bacc("TRN2", target_bir_lowering=False, debug=False,
                   num_devices=NCORE)

    dmap = {np.dtype(np.float32): dt.float32, np.dtype(BF16): dt.bfloat16,
            np.dtype(np.int16): dt.int16, np.dtype(F8): dt.float8e4}
    I = {}
    for d in (com, pc0):
        for k, a in d.items():
            I[k] = nc.dram_tensor(k, list(a.shape), dmap[a.dtype],
                                  kind="ExternalInput")

    out_t = nc.dram_tensor("out", [OUT, G], dt.float32, kind="ExternalOutput")
    NBK = meta["NBK"]
    H1 = c["H1"]
    dbg = dict(
        rden=nc.dram_tensor("dbg_rden", [NBK, 128, 1], dt.float32),
        es=nc.dram_tensor("dbg_es", [NBK, 128, 1], dt.float32),
        usb=nc.dram_tensor("dbg_usb", [NBK, 128, 128], dt.bfloat16),
        t2=nc.dram_tensor("dbg_t2", [NBK, 128, H1], dt.float32),
        e4=nc.dram_tensor("dbg_e4", [NBK, 128, 2], dt.float32),
    )
    ag2_in = nc.dram_tensor("ag2_in", [BPC, H2], dt.bfloat16)
    tbl2 = nc.dram_tensor("tbl2", [NPAD2, H2], dt.bfloat16, addr_space="Shared")
    pool_in = nc.dram_tensor("pool_in", [G, H2], dt.float32)
    pool_out = nc.dram_tensor("pool_out", [G, H2], dt.float32, addr_space="Shared")

    with tile.TileContext(nc) as tc:
        _body(nc, tc, I, out_t, ag2_in, tbl2, pool_in, pool_out,
              meta, bass, tile, mybir, library_config, dbg=dbg)
    nc.compile()
    return nc


DEBUG = False


def _body(nc, tc, I, out_t, ag2_in, tbl2, pool_in, pool_out,
          meta, bass, tile, mybir, library_config, dbg=None):
    from contextlib import ExitStack

    c = meta["cfg"]
    G = c["G"]
    DIN, ED, H1, H2, HD, OUT = c["DIN"], c["ED"], c["H1"], c["H2"], c["HD"], c["OUT"]
    NCORE, HALF = c["NC"], c["HALF"]
    NPC, NBK, BPC = meta["NPC"], meta["NBK"], meta["BPC"]
    NPAD1, NPAD2 = meta["NPAD1"], meta["NPAD2"]
    P1, P2 = meta["P1"], meta["P2"]
    CHMX = meta["CHMX"]
    AF = mybir.ActivationFunctionType
    dt = mybir.dt
    Alu = mybir.AluOpType
    ds = bass.ds

    nc.gpsimd.load_library(library_config.mlp)
    pid = nc.partition_id()

    ctx = ExitStack()
    with ctx:
        consts = ctx.enter_context(tc.tile_pool(name="consts", bufs=1))

        def cload(name):
            a = I[name]
            t = consts.tile(list(a.shape), a.dtype, tag=name)
            nc.sync.dma_start(t[:], a[:])
            return t

        iota_col = cload("iota_col")
        IOTAF4 = cload("IOTAF4")
        IDENT = cload("IDENT")
        IDENT32 = cload("IDENT32")
        ones_col = cload("ones_col")
        Wl1p = cload("Wl1p"); Wr1p = cload("Wr1p"); We1p = cload("We1p")
        Wl2pp = cload("Wl2pp"); Wr2pp = cload("Wr2pp"); We2p = cload("We2p")
        brB1 = cload("brB1"); bB1 = cload("bB1")
        brB2 = cload("brB2"); bB2 = cload("bB2")
        natT = cload("natT"); na_own = cload("na_own"); laT = cload("laT")

        res = ctx.enter_context(tc.tile_pool(name="res", bufs=1))
        xr1_nm = res.tile([128, NBK, H1], dt.bfloat16, tag="xr1")
        x1_T = res.tile([128, H1 // 128, BPC], dt.bfloat16, tag="x1T")
        xr2_nm = res.tile([128, NBK, H2], dt.bfloat16, tag="xr2")

        # ---------------- phase 0: xr1 for own nodes -------------------
        with tc.tile_pool(name="p0ps", bufs=2, space="PSUM") as p0ps:
            for b in range(NBK):
                ps = p0ps.tile([128, H1], dt.float32, tag="xr1ps")
                nc.tensor.matmul(ps[:], natT[:, b, :], Wr1p[:],
                                 start=True, stop=True)
                nc.vector.tensor_tensor(xr1_nm[:, b, :], ps[:], brB1[:], op=Alu.add)

        # ---------------- shared pools for both edge phases ------------
        sb = ctx.enter_context(tc.tile_pool(name="sb", bufs=4))
        sbg = ctx.enter_context(tc.tile_pool(name="sbg", bufs=4))
        pre = ctx.enter_context(tc.tile_pool(name="pre", bufs=1))



        # ---------------- shared edge phase ----------------------------
        def edge_phase(lay, ps_s4, ps_tr, ps_U, ps_den, ps_misc,
                       pool_ps=None, PT_sb=None):
            H = H1 if lay == 1 else H2
            Ppos = P1 if lay == 1 else P2
            We = We1p if lay == 1 else We2p
            xr_nm = xr1_nm if lay == 1 else xr2_nm
            bB = bB1 if lay == 1 else bB2
            sfx = str(lay)
            Ls = meta["L" + sfx]
            seg, offs, nmax = Ls["seg"], Ls["offs"], Ls["nmax"]
            L = int(Ls["L"]); C = int(Ls["C"])
            if lay == 1:
                tlo = I["na_bf"][0:HALF, :]
                thi = I["na_bf"][HALF:NPAD1, :]
            else:
                tlo = tbl2[0:HALF, :]
                thi = tbl2[HALF:NPAD2, :]

            idx_all = pre.tile([128, L // 16], dt.int16, tag="idx" + sfx)
            nc.sync.dma_start(idx_all[:], I["idxs" + sfx][:])
            dlc_all = pre.tile([128, C], dt.float32, tag="dlc" + sfx)
            nc.scalar.dma_start(dlc_all[:], I["dlc" + sfx][:])

            for b in range(NBK):
                if lay == 1:
                    U_T = ps_U.tile([128, 128], dt.float32, tag="UT")
                    den = ps_den.tile([128, 8], dt.float32, tag="den")
                else:
                    agg = ps_U.tile([128, H2 + 8], dt.float32, tag="agg2")
                    xlw = sbg.tile([128, H2], dt.bfloat16, tag="xlw")
                    nc.sync.dma_start(xlw[:], tbl2[ds(pid * BPC + b * 128, 128), :])
                first = True
                cidx0 = int(offs[b, 0]) // 128
                for h in range(2):
                    sl = int(seg[b, h]); o = int(offs[b, h])
                    if sl == 0:
                        continue
                    nch_all = sl // 128
                    xlg = sbg.tile([128, nch_all, 128], dt.bfloat16, tag="xlg")
                    nm = int(nmax[b, h])
                    if nm < sl:
                        # slots [nm:sl) are skipped by the gather (trailing
                        # negative idxs) and would hold stale SBUF data; zero
                        # the last chunk first so downstream exp/matmuls see
                        # finite values (the gather overwrites real rows).
                        nc.vector.memset(xlg[:, nch_all - 1, :], 0.0)
                    nc.gpsimd.dma_gather(xlg[:], thi if h else tlo,
                                         idx_all[:, o // 16:(o + sl) // 16],
                                         sl, nm, 128)
                    eTs = sb.tile([32, CHMX * 128], dt.bfloat16, tag="eT")
                    nc.sync.dma_start(eTs[:, :sl], I["eT" + sfx][:, o:o + sl])
                    M8s = sb.tile([128, CHMX * 128], dt.float8e4, tag="M8")
                    nc.scalar.dma_start(M8s[:, :sl], I["M8" + sfx][:, o:o + sl])
                    for po in range(0, sl, 256):
                        pl = min(256, sl - po)
                        nch = pl // 128
                        jj0 = (o + po) // 128
                        s4 = ps_s4.tile([128, 2, H], dt.float32, tag="s4")
                        for j in range(nch):
                            cs = xlg[:, po // 128 + j, :]
                            ec = slice(po + j * 128, po + (j + 1) * 128)
                            if lay == 1:
                                tp = ps_tr.tile([128, 128], dt.bfloat16, tag="tr")
                                nc.tensor.transpose(tp[:], cs, IDENT[:])
                                nagT = sb.tile([128, 128], dt.bfloat16, tag="nagT")
                                nc.vector.tensor_copy(nagT[:], tp[:])
                                nc.tensor.matmul(s4[:, j, :], nagT[:], Wl1p[:],
                                                 start=(j == 0), stop=False)
                            else:
                                nc.tensor.matmul(s4[:, j, :], IDENT[:], cs,
                                                 start=(j == 0), stop=False)
                            nc.tensor.matmul(s4[:, j, :], M8s[:, ec], xr_nm[:, b, :],
                                             start=False, stop=False)
                            nc.tensor.matmul(s4[:, j, :], eTs[:, ec], We[:],
                                             start=False, stop=(j == nch - 1))
                        ls4 = sb.tile([128, 2, H], dt.bfloat16, tag="ls4")
                        if Ppos > 0:
                            nc.scalar.activation(ls4[:, :nch, 0:Ppos], s4[:, :nch, 0:Ppos],
                                                 AF.Prelu, alpha=0.2)
                        if Ppos < H:
                            nc.scalar.activation(ls4[:, :nch, Ppos:H], s4[:, :nch, Ppos:H],
                                                 AF.Prelu, scale=-0.2, alpha=5.0)
                        e4 = sb.tile([128, 2], dt.float32, tag="e4")
                        nc.vector.reduce_sum(e4[:, :nch], ls4[:, :nch, :],
                                             axis=mybir.AxisListType.X)
                        w4 = sb.tile([128, 2], dt.float32, tag="w4")
                        nc.scalar.activation(w4[:, :nch], e4[:, :nch], AF.Exp)
                        if DEBUG and lay == 1 and h == 0 and po == 0:
                            nc.sync.dma_start(dbg["e4"][b], e4[:])
                        for j in range(nch):
                            cs = xlg[:, po // 128 + j, :]
                            MwT = sb.tile([128, 128], dt.bfloat16, tag="mwt")
                            nc.vector.tensor_scalar(
                                MwT[:], IOTAF4[:, j * 128:(j + 1) * 128],
                                dlc_all[:, jj0 + j:jj0 + j + 1], w4[:, j:j + 1],
                                op0=Alu.is_equal, op1=Alu.mult)
                            if lay == 1:
                                nc.tensor.matmul(U_T[:], cs, MwT[:],
                                                 start=first, stop=False)
                                nc.tensor.matmul(den[:, 0:1], MwT[:], ones_col[:],
                                                 start=first, stop=False)
                            else:
                                nc.tensor.matmul(agg[:, 0:H2], MwT[:], cs,
                                                 start=first, stop=False)
                                nc.tensor.matmul(agg[:, H2:H2 + 1], MwT[:], ones_col[:],
                                                 start=False, stop=False)
                            first = False
                # ---------------- self-loop chunk -----------------------
                s_s = ps_s4.tile([128, 2, H], dt.float32, tag="s4")
                if lay == 1:
                    nc.tensor.matmul(s_s[:, 0, :], natT[:, b, :], Wl1p[:],
                                     start=True, stop=False)
                else:
                    nc.tensor.matmul(s_s[:, 0, :], IDENT[:], xlw[:],
                                     start=True, stop=False)
                nc.tensor.matmul(s_s[:, 0, :], IDENT[:], xr_nm[:, b, :],
                                 start=False, stop=False)
                nc.tensor.matmul(s_s[:, 0, :], laT[:, b, :], We[:],
                                 start=False, stop=True)
                ls_s = sb.tile([128, 2, H], dt.bfloat16, tag="ls4")
                if Ppos > 0:
                    nc.scalar.activation(ls_s[:, 0, 0:Ppos], s_s[:, 0, 0:Ppos],
                                         AF.Prelu, alpha=0.2)
                if Ppos < H:
                    nc.scalar.activation(ls_s[:, 0, Ppos:H], s_s[:, 0, Ppos:H],
                                         AF.Prelu, scale=-0.2, alpha=5.0)
                es = sb.tile([128, 1], dt.float32, tag="es")
                nc.vector.reduce_sum(es[:], ls_s[:, 0:1, :], axis=mybir.AxisListType.X)
                ws = sb.tile([128, 1], dt.float32, tag="ws")
                nc.scalar.activation(ws[:], es[:], AF.Exp)
                if DEBUG and lay == 1:
                    nc.sync.dma_start(dbg["es"][b], es[:])
                diag = sb.tile([128, 128], dt.bfloat16, tag="mwt")
                nc.vector.tensor_scalar(diag[:], IOTAF4[:, 0:128], iota_col[:],
                                        ws[:], op0=Alu.is_equal, op1=Alu.mult)
                if lay == 1:
                    nc.tensor.matmul(U_T[:], na_own[:, b, :], diag[:],
                                     start=False, stop=True)
                    nc.tensor.matmul(den[:, 0:1], diag[:], ones_col[:],
                                     start=False, stop=True)
                else:
                    nc.tensor.matmul(agg[:, 0:H2], diag[:], xlw[:],
                                     start=False, stop=False)
                    nc.tensor.matmul(agg[:, H2:H2 + 1], diag[:], ones_col[:],
                                     start=False, stop=True)
                # ---------------- block finalize ------------------------
                if lay == 1:
                    U_sb = sb.tile([128, 128], dt.bfloat16, tag="usb")
                    nc.vector.tensor_copy(U_sb[:], U_T[:])
                    aggf = ps_misc.tile([128, H1], dt.float32, tag="misc")
                    nc.tensor.matmul(aggf[:], U_sb[:], Wl1p[:], start=True, stop=True)
                    rden = sb.tile([128, 1], dt.float32, tag="rden")
                    nc.vector.reciprocal(rden[:], den[:, 0:1])
                    if DEBUG:
                        nc.sync.dma_start(dbg["usb"][b], U_sb[:])
                        nc.sync.dma_start(dbg["rden"][b], rden[:])
                    aggp, denc = aggf, rden
                else:
                    rden = sb.tile([128, 1], dt.float32, tag="rden")
                    nc.vector.reciprocal(rden[:], agg[:, H2:H2 + 1])
                    aggp, denc = agg, rden
                t1 = sb.tile([128, H], dt.float32, tag="t1")
                nc.vector.tensor_scalar(t1[:], aggp[:, 0:H], denc[:], None, op0=Alu.mult)
                t2 = sb.tile([128, H], dt.float32, tag="t2")
                nc.vector.tensor_tensor(t2[:], t1[:], bB[:], op=Alu.add)
                if DEBUG and lay == 1:
                    nc.sync.dma_start(dbg["t2"][b], t2[:])
                x_nm = sb.tile([128, H], dt.bfloat16, tag="xnm")
                nc.scalar.activation(x_nm[:], t2[:], AF.Relu)
                if lay == 1:
                    for hh in range(H1 // 128):
                        tp = ps_tr.tile([128, 128], dt.bfloat16, tag="tr")
                        nc.tensor.transpose(tp[:], x_nm[:, hh * 128:(hh + 1) * 128], IDENT[:])
                        nc.scalar.copy(x1_T[:, hh, b * 128:(b + 1) * 128], tp[:])
                    psl = ps_misc.tile([128, H2], dt.float32, tag="misc")
                    for hh in range(H1 // 128):
                        nc.tensor.matmul(psl[:], x1_T[:, hh, b * 128:(b + 1) * 128],
                                         Wl2pp[:, hh * H2:(hh + 1) * H2],
                                         start=(hh == 0), stop=(hh == H1 // 128 - 1))
                    sbx = sb.tile([128, H2], dt.bfloat16, tag="sbx")
                    nc.vector.tensor_copy(sbx[:], psl[:])
                    nc.sync.dma_start(ag2_in[b * 128:(b + 1) * 128, :], sbx[:])
                    psr = ps_misc.tile([128, H2], dt.float32, tag="misc")
                    for hh in range(H1 // 128):
                        nc.tensor.matmul(psr[:], x1_T[:, hh, b * 128:(b + 1) * 128],
                                         Wr2pp[:, hh * H2:(hh + 1) * H2],
                                         start=(hh == 0), stop=(hh == H1 // 128 - 1))
                    nc.vector.tensor_tensor(xr2_nm[:, b, :], psr[:], brB2[:], op=Alu.add)
                else:
                    nc.tensor.matmul(pool_ps[:, 0:H2], PT_sb[b][:], x_nm[:],
                                     start=(b == 0), stop=(b == NBK - 1))

        # layer-1 edge phase
        with ExitStack() as ctx1:
            ps_s4 = ctx1.enter_context(tc.tile_pool(name="ps_s4", bufs=2, space="PSUM"))
            ps_tr = ctx1.enter_context(tc.tile_pool(name="ps_tr", bufs=1, space="PSUM"))
            ps_U = ctx1.enter_context(tc.tile_pool(name="ps_U", bufs=2, space="PSUM"))
            ps_den = ctx1.enter_context(tc.tile_pool(name="ps_den", bufs=1, space="PSUM"))
            ps_misc = ctx1.enter_context(tc.tile_pool(name="ps_misc", bufs=2, space="PSUM"))
            edge_phase(1, ps_s4, ps_tr, ps_U, ps_den, ps_misc)

        nc.gpsimd.collective_compute(
            "AllGather", mybir.AluOpType.bypass,
            replica_groups=[list(range(NCORE))],
            ins=[ag2_in[:]], outs=[tbl2[:]])

        # ---------------- layer-2 edge phase + pooling ------------------
        pool_pp = ctx.enter_context(tc.tile_pool(name="poolps", bufs=1, space="PSUM"))
        pool_ps = pool_pp.tile([G, H2 + 4], dt.float32, tag="pool")
        pt_pool = ctx.enter_context(tc.tile_pool(name="ptsb", bufs=1))
        PT_sb = []
        for b in range(NBK):
            t = pt_pool.tile([128, G], dt.bfloat16, tag=f"pt{b}")
            nc.sync.dma_start(t[:], I["PT"][b])
            PT_sb.append(t)
        with ExitStack() as ctx2:
            ps_s4 = ctx2.enter_context(tc.tile_pool(name="ps_s4b", bufs=2, space="PSUM"))
            ps_tr = ctx2.enter_context(tc.tile_pool(name="ps_trb", bufs=1, space="PSUM"))
            ps_U = ctx2.enter_context(tc.tile_pool(name="ps_Ub", bufs=2, space="PSUM"))
            ps_den = ctx2.enter_context(tc.tile_pool(name="ps_denb", bufs=1, space="PSUM"))
            ps_misc = ctx2.enter_context(tc.tile_pool(name="ps_miscb", bufs=2, space="PSUM"))
            edge_phase(2, ps_s4, ps_tr, ps_U, ps_den, ps_misc,
                       pool_ps=pool_ps, PT_sb=PT_sb)

        # ---------------- head -----------------------------------------
        with tc.tile_pool(name="hsb", bufs=2) as hsb, \
             tc.tile_pool(name="hps", bufs=2, space="PSUM") as hps:
            psb = hsb.tile([G, H2], dt.float32, tag="poolsb")
            nc.scalar.copy(psb[:], pool_ps[:, 0:H2])
            nc.sync.dma_start(pool_in[:], psb[:])
            nc.gpsimd.collective_compute(
                "AllReduce", mybir.AluOpType.add,
                replica_groups=[list(range(NCORE))],
                ins=[pool_in[:]], outs=[pool_out[:]])
            pooled = hsb.tile([G, H2], dt.float32, tag="pooled")
            nc.sync.dma_start(pooled[:], pool_out[:])
            pooled_T_ps = hps.tile([H2, G], dt.float32, tag="pooledT")
            nc.tensor.transpose(pooled_T_ps[:], pooled[:], IDENT32[0:G, 0:G])
            pooled_T = hsb.tile([H2, G], dt.float32, tag="pooledTsb")
            nc.scalar.copy(pooled_T[:], pooled_T_ps[:])
            Wd1sb = hsb.tile([H2, HD], dt.float32, tag="wd1")
            nc.sync.dma_start(Wd1sb[:], I["Wd1u"][:])
            h1ps = hps.tile([HD, G], dt.float32, tag="h1")
            nc.tensor.matmul(h1ps[:], Wd1sb[:], pooled_T[:], start=True, stop=True)
            hscale = hsb.tile([HD, 1], dt.float32, tag="hscale")
            nc.sync.dma_start(hscale[:], I["head_scale"][:])
            hbias = hsb.tile([HD, 1], dt.float32, tag="hbias")
            nc.sync.dma_start(hbias[:], I["head_bias"][:])
            th = hsb.tile([HD, G], dt.float32, tag="th")
            nc.scalar.activation(th[:], h1ps[:], AF.Prelu, bias=hbias[:],
                                 scale=hscale[:], alpha=0.1)
            Wd2sb = hsb.tile([HD, OUT], dt.float32, tag="wd2")
            nc.sync.dma_start(Wd2sb[:], I["Wd2"][:])
            ops = hps.tile([OUT, G], dt.float32, tag="ops")
            nc.tensor.matmul(ops[:], Wd2sb[:], th[:], start=True, stop=True)
            bd2sb = hsb.tile([OUT, 1], dt.float32, tag="bd2sb")
            nc.sync.dma_start(bd2sb[:], I["bd2"][:])
            osb = hsb.tile([OUT, G], dt.float32, tag="osb")
            nc.vector.tensor_scalar(osb[:], ops[:], bd2sb[:], None, op0=Alu.add)
            nc.sync.dma_start(out_t[:], osb[:])


def _kernel(inputs, cfg, runner=None, trace=False):
    com, percore, meta = host_prep(inputs, cfg)
    nc = build_program(meta, com, percore[0])
    in_maps = [dict(com, **pc) for pc in percore]
    if runner is None:
        from concourse.bass_utils import run_bass_kernel_spmd
        res = run_bass_kernel_spmd(nc, in_maps, list(range(cfg["NC"])), trace=trace)
        out = np.asarray(res.results[0]["out"])
        return out.T.copy().astype(np.float32), res
    return runner(nc, in_maps)


def kernel(**inputs):
    out, _ = _kernel(inputs, DEFAULT_CFG)
    return out


# revision 25
# speedup vs baseline: 1.2070x; 1.1422x over previous
"""GATv2 x2 + global-mean-pool + MLP head on 8 NeuronCores (Bass/Tile).

Sharding: destination-partitioned. Core c owns nodes [c*NPC, (c+1)*NPC);
it processes every edge whose dst is in its range, so attention softmax
segments are core-local.

Layer 1 gathers RAW node_attr rows (256B) per edge — no xl1 table is
ever materialized.  The per-chunk score xl-term is a matmul of the
transposed gathered rows with Wl1; the aggregation accumulates
U_T[DIN, d] += NA_g.T @ MwT per chunk and applies Wl1 once per block.
Layer 2 AllGathers the raw xl2 table (x1 @ Wl2, no bias) and gathers
its 256B rows per edge.

Host precomputes: loop_attr (self-loop edge features), per-chunk dst
one-hot matrices M [dstrow, edge] (streamed, fp8), dst-local-row
columns, and folds |att| into the weights (channels permuted so
positive-att channels come first; see baseline notes).  All biases are
folded: the score-side bias (bl+br) rides on xr; the output-side bias
(bl+b) is added at block finalize (valid since softmax weights sum to
1).  exp is applied without max-subtraction: logits are O(1) here.
"""

import sys
import numpy as np
import ml_dtypes

sys.path.insert(0, "/opt/trn_rl_repo")

BF16 = ml_dtypes.bfloat16
F8 = ml_dtypes.float8_e4m3

DEFAULT_CFG = dict(
    N=50000, E=500000, G=64,
    DIN=128, ED=32, H1=256, H2=128, HD=64, OUT=8,
    NC=8, HALF=32768,
)


def _roundup(x, m):
    return (x + m - 1) // m * m


def _wrap16(idx, L):
    out = np.full((128, max(L // 16, 1)), -1, np.int16)
    n = len(idx)
    if n:
        pos = np.arange(n)
        out[pos % 16, pos // 16] = idx.astype(np.int16)
    for g in range(1, 8):
        out[g * 16:(g + 1) * 16] = out[0:16]
    return out


def host_prep(inputs, cfg):
    c = dict(cfg)
    N, E, G = c["N"], c["E"], c["G"]
    DIN, ED, H1, H2 = c["DIN"], c["ED"], c["H1"], c["H2"]
    NCORE, HALF = c["NC"], c["HALF"]
    NPC = N // NCORE
    NBK = _roundup(NPC, 128) // 128
    BPC = NBK * 128
    NPAD1 = _roundup(N, 512)
    NPAD2 = NCORE * BPC

    f64 = lambda x: np.asarray(x, np.float64)
    att1, att2 = f64(inputs["att1"]), f64(inputs["att2"])
    a1 = np.maximum(np.abs(att1), 1e-12); s1 = np.where(att1 >= 0, 1.0, -1.0)
    a2 = np.maximum(np.abs(att2), 1e-12); s2 = np.where(att2 >= 0, 1.0, -1.0)
    perm1 = np.argsort(-s1, kind="stable"); P1 = int((s1 > 0).sum())
    perm2 = np.argsort(-s2, kind="stable"); P2 = int((s2 > 0).sum())
    a1p, a2p = a1[perm1], a2[perm2]

    Wl1p = (f64(inputs["Wl1"]) * a1)[:, perm1]
    Wr1p = (f64(inputs["Wr1"]) * a1)[:, perm1]
    We1p = (f64(inputs["We1"]) * a1)[:, perm1]
    bl1p = (f64(inputs["bl1"]) * a1)[perm1]
    br1p = (f64(inputs["br1"]) * a1)[perm1]
    b1p = (f64(inputs["b1"]) * a1)[perm1]

    Wl2u = f64(inputs["Wl2"])[perm1, :] / a1p[:, None]
    Wr2u = f64(inputs["Wr2"])[perm1, :] / a1p[:, None]
    Wl2pp = (Wl2u * a2)[:, perm2]
    Wr2pp = (Wr2u * a2)[:, perm2]
    We2p = (f64(inputs["We2"]) * a2)[:, perm2]
    bl2p = (f64(inputs["bl2"]) * a2)[perm2]
    br2p = (f64(inputs["br2"]) * a2)[perm2]
    b2p = (f64(inputs["b2"]) * a2)[perm2]

    Wd1u = f64(inputs["Wd1"])[perm2, :] / a2p[:, None]
    bs = f64(inputs["bn_gamma"]) / np.sqrt(f64(inputs["bn_var"]) + 1e-5)
    head_scale = bs
    head_bias = (f64(inputs["bd1"]) * bs + f64(inputs["bn_beta"])
                 - f64(inputs["bn_mean"]) * bs)

    src = np.asarray(inputs["edge_src"], np.int64)
    dst = np.asarray(inputs["edge_dst"], np.int64)
    batch = np.asarray(inputs["batch"], np.int64)
    eattr = np.asarray(inputs["edge_attr"], np.float64)

    # loop_attr (self-loop edge features) on host: segment mean of eattr by dst
    deg = np.bincount(dst, minlength=N).astype(np.float64)
    order_d = np.argsort(dst, kind="stable")
    eattr_sorted = eattr[order_d]
    cuts = np.searchsorted(dst[order_d], np.arange(N))
    la = np.zeros((N, ED), np.float64)
    nz = deg > 0
    sums = np.add.reduceat(eattr_sorted, np.minimum(cuts, len(dst) - 1), axis=0)
    la[nz] = sums[nz] / deg[nz][:, None]

    core_of = dst // NPC
    blk_of = (dst % NPC) // 128
    dloc_of = (dst % NPC) % 128

    def layer_streams(row):
        half = (row >= HALF).astype(np.int64)
        cnt = np.zeros((NCORE, NBK, 2), np.int64)
        np.add.at(cnt, (core_of, blk_of, half), 1)
        nmax = cnt.max(axis=0)                         # [NBK, 2] real rows
        nmax[:, 0] = np.maximum(nmax[:, 0], 1)
        seg = _roundup(nmax, 128)                      # [NBK, 2]
        seg[:, 0] = np.maximum(seg[:, 0], 128)
        offs = np.zeros((NBK, 2), np.int64)
        L = 0
        for b in range(NBK):
            for h in range(2):
                offs[b, h] = L
                L += seg[b, h]
        C = L // 128
        key = core_of * (NBK * 2) + blk_of * 2 + half
        order = np.argsort(key, kind="stable")
        ks = key[order]
        idxs = np.zeros((NCORE, 128, L // 16), np.int16)
        eT = np.zeros((NCORE, ED, L), BF16)
        M8 = np.zeros((NCORE, 128, L), F8)
        dlc = np.full((NCORE, 128, C), 200.0, np.float32)
        bounds = np.searchsorted(ks, np.arange(NCORE * NBK * 2 + 1))
        for cr in range(NCORE):
            for b in range(NBK):
                for h in range(2):
                    k = cr * (NBK * 2) + b * 2 + h
                    m = order[bounds[k]:bounds[k + 1]]
                    n = len(m)
                    o = int(offs[b, h]); sl = int(seg[b, h])
                    if sl == 0:
                        continue
                    nm = int(nmax[b, h])
                    loc_idx = np.full(sl, -1, np.int64)
                    loc_idx[:nm] = 0
                    loc_idx[:n] = row[m] - h * HALF
                    idxs[cr][:, o // 16:(o + sl) // 16] = _wrap16(loc_idx, sl)
                    if n:
                        eT[cr][:, o:o + n] = eattr[m].T.astype(BF16)
                        M8[cr][dloc_of[m], o + np.arange(n)] = F8(1.0)
                        p = np.arange(n)
                        dlc[cr][p % 128, o // 128 + p // 128] = dloc_of[m]
        return dict(seg=seg, offs=offs, nmax=nmax, L=L, C=C, idxs=idxs, eT=eT,
                    M8=M8, dlc=dlc)

    row1 = src
    row2 = BPC * (src // NPC) + (src % NPC)
    L1s = layer_streams(row1)
    L2s = layer_streams(row2)
    CHMX = int(max(L1s["seg"].max(), L2s["seg"].max())) // 128

    cnts = np.maximum(np.bincount(batch, minlength=G).astype(np.float64), 1.0)
    PT = np.zeros((NCORE, NBK, 128, G), BF16)
    for cr in range(NCORE):
        for b in range(NBK):
            base = cr * NPC + b * 128
            nn = min(128, NPC - b * 128)
            if nn <= 0:
                continue
            gids = batch[base:base + nn]
            PT[cr, b, np.arange(nn), gids] = (1.0 / cnts[gids]).astype(BF16)

    iota_col = np.arange(128, dtype=np.float32).reshape(128, 1)
    IOTAF4 = np.tile(np.arange(128, dtype=np.float32)[None, :], (128, 4))
    IDENT = np.eye(128, dtype=BF16)
    IDENT32 = np.eye(128, dtype=np.float32)
    ones_col = np.ones((128, 1), BF16)

    na_bf = np.zeros((NPAD1, DIN), BF16)
    na_f32 = np.asarray(inputs["node_attr"], np.float32)
    na_bf[:N] = na_f32.astype(BF16)

    # per-core own-node views (self-loop chunks + xr transform)
    natT = np.zeros((NCORE, DIN, NBK, 128), BF16)
    na_own = np.zeros((NCORE, 128, NBK, DIN), BF16)
    laT = np.zeros((NCORE, ED, NBK, 128), BF16)
    for cr in range(NCORE):
        for b in range(NBK):
            base = cr * NPC + b * 128
            nn = min(128, N - base) if base < N else 0
            nn = min(nn, NPC - b * 128)
            if nn <= 0:
                continue
            natT[cr, :, b, :nn] = na_f32[base:base + nn].T.astype(BF16)
            na_own[cr, :nn, b, :] = na_f32[base:base + nn].astype(BF16)
            laT[cr, :, b, :nn] = la[base:base + nn].T.astype(BF16)

    bcast = lambda v: np.tile(np.asarray(v, np.float32)[None, :], (128, 1)).copy()

    com = dict(
        na_bf=na_bf,
        Wl1p=Wl1p.astype(BF16), Wr1p=Wr1p.astype(BF16), We1p=We1p.astype(BF16),
        Wl2pp=Wl2pp.reshape(H1 // 128, 128, H2).transpose(1, 0, 2).reshape(128, -1).astype(BF16),
        Wr2pp=Wr2pp.reshape(H1 // 128, 128, H2).transpose(1, 0, 2).reshape(128, -1).astype(BF16),
        We2p=We2p.astype(BF16),
        brB1=bcast(bl1p + br1p), bB1=bcast(bl1p + b1p),
        brB2=bcast(bl2p + br2p), bB2=bcast(bl2p + b2p),
        Wd1u=Wd1u.astype(np.float32),
        head_scale=head_scale.astype(np.float32).reshape(-1, 1),
        head_bias=head_bias.astype(np.float32).reshape(-1, 1),
        Wd2=np.asarray(inputs["Wd2"], np.float32),
        bd2=np.asarray(inputs["bd2"], np.float32).reshape(-1, 1),
        iota_col=iota_col, IOTAF4=IOTAF4, IDENT=IDENT, IDENT32=IDENT32,
        ones_col=ones_col,
    )
    percore = []
    for cr in range(NCORE):
        percore.append(dict(
            idxs1=L1s["idxs"][cr], eT1=L1s["eT"][cr], M81=L1s["M8"][cr],
            dlc1=L1s["dlc"][cr],
            idxs2=L2s["idxs"][cr], eT2=L2s["eT"][cr], M82=L2s["M8"][cr],
            dlc2=L2s["dlc"][cr],
            PT=PT[cr], natT=natT[cr], na_own=na_own[cr], laT=laT[cr],
        ))
    meta = dict(cfg=c, NPC=NPC, NBK=NBK, BPC=BPC, NPAD1=NPAD1, NPAD2=NPAD2,
                P1=P1, P2=P2, L1=L1s, L2=L2s, CHMX=CHMX)
    return com, percore, meta


def build_program(meta, com, pc0):
    import concourse.bass as bass
    import concourse.tile as tile
    from concourse import bacc, mybir
    from concourse import library_config

    c = meta["cfg"]
    G, H2, OUT = c["G"], c["H2"], c["OUT"]
    NCORE = c["NC"]
    BPC = meta["BPC"]
    NPAD2 = meta["NPAD2"]
    dt = mybir.dt

    nc = bacc.Bacc("TRN2", target_bir_lowering=False, debug=False,
                   num_devices=NCORE)

    dmap = {np.dtype(np.float32): dt.float32, np.dtype(BF16): dt.bfloat16,
            np.dtype(np.int16): dt.int16, np.dtype(F8): dt.float8e4}
    I = {}
    for d in (com, pc0):
        for k, a in d.items():
            I[k] = nc.dram_tensor(k, list(a.shape), dmap[a.dtype],
                                  kind="ExternalInput")

    out_t = nc.dram_tensor("out", [OUT, G], dt.float32, kind="ExternalOutput")
    NBK = meta["NBK"]
    H1 = c["H1"]
    dbg = dict(
        rden=nc.dram_tensor("dbg_rden", [NBK, 128, 1], dt.float32),
        es=nc.dram_tensor("dbg_es", [NBK, 128, 1], dt.float32),
        usb=nc.dram_tensor("dbg_usb", [NBK, 128, 128], dt.bfloat16),
        t2=nc.dram_tensor("dbg_t2", [NBK, 128, H1], dt.float32),
        e4=nc.dram_tensor("dbg_e4", [NBK, 128, 2], dt.float32),
    )
    ag2_in = nc.dram_tensor("ag2_in", [BPC, H2], dt.bfloat16)
    tbl2 = nc.dram_tensor("tbl2", [NPAD2, H2], dt.bfloat16, addr_space="Shared")
    pool_in = nc.dram_tensor("pool_in", [G, H2], dt.float32)
    pool_out = nc.dram_tensor("pool_out", [G, H2], dt.float32, addr_space="Shared")

    with tile.TileContext(nc) as tc:
        _body(nc, tc, I, out_t, ag2_in, tbl2, pool_in, pool_out,
              meta, bass, tile, mybir, library_config, dbg=dbg)
    nc.compile()
    return nc


DEBUG = False


def _body(nc, tc, I, out_t, ag2_in, tbl2, pool_in, pool_out,
          meta, bass, tile, mybir, library_config, dbg=None):
    from contextlib import ExitStack

    c = meta["cfg"]
    G = c["G"]
    DIN, ED, H1, H2, HD, OUT = c["DIN"], c["ED"], c["H1"], c["H2"], c["HD"], c["OUT"]
    NCORE, HALF = c["NC"], c["HALF"]
    NPC, NBK, BPC = meta["NPC"], meta["NBK"], meta["BPC"]
    NPAD1, NPAD2 = meta["NPAD1"], meta["NPAD2"]
    P1, P2 = meta["P1"], meta["P2"]
    CHMX = meta["CHMX"]
    AF = mybir.ActivationFunctionType
    dt = mybir.dt
    Alu = mybir.AluOpType
    ds = bass.ds

    nc.gpsimd.load_library(library_config.mlp)
    pid = nc.partition_id()

    ctx = ExitStack()
    with ctx:
        consts = ctx.enter_context(tc.tile_pool(name="consts", bufs=1))

        def cload(name):
            a = I[name]
            t = consts.tile(list(a.shape), a.dtype, tag=name)
            nc.sync.dma_start(t[:], a[:])
            return t

        iota_col = cload("iota_col")
        IOTAF4 = cload("IOTAF4")
        IDENT = cload("IDENT")
        IDENT32 = cload("IDENT32")
        ones_col = cload("ones_col")
        Wl1p = cload("Wl1p"); Wr1p = cload("Wr1p"); We1p = cload("We1p")
        Wl2pp = cload("Wl2pp"); Wr2pp = cload("Wr2pp"); We2p = cload("We2p")
        brB1 = cload("brB1"); bB1 = cload("bB1")
        brB2 = cload("brB2"); bB2 = cload("bB2")
        natT = cload("natT"); na_own = cload("na_own"); laT = cload("laT")

        res = ctx.enter_context(tc.tile_pool(name="res", bufs=1))
        xr1_nm = res.tile([128, NBK, H1], dt.bfloat16, tag="xr1")
        x1_T = res.tile([128, H1 // 128, BPC], dt.bfloat16, tag="x1T")
        xr2_nm = res.tile([128, NBK, H2], dt.bfloat16, tag="xr2")

        # ---------------- phase 0: xr1 for own nodes -------------------
        with tc.tile_pool(name="p0ps", bufs=2, space="PSUM") as p0ps:
            for b in range(NBK):
                ps = p0ps.tile([128, H1], dt.float32, tag="xr1ps")
                nc.tensor.matmul(ps[:], natT[:, b, :], Wr1p[:],
                                 start=True, stop=True)
                nc.vector.tensor_tensor(xr1_nm[:, b, :], ps[:], brB1[:], op=Alu.add)

        # ---------------- shared pools for both edge phases ------------
        sb = ctx.enter_context(tc.tile_pool(name="sb", bufs=4))
        sbg = ctx.enter_context(tc.tile_pool(name="sbg", bufs=5))
        nag = ctx.enter_context(tc.tile_pool(name="nag", bufs=CHMX + 4))
        pre = ctx.enter_context(tc.tile_pool(name="pre", bufs=1))



        # ---------------- shared edge phase ----------------------------
        def edge_phase(lay, ps_s4, ps_tr, ps_U, ps_den, ps_misc,
                       pool_ps=None, PT_sb=None):
            H = H1 if lay == 1 else H2
            Ppos = P1 if lay == 1 else P2
            We = We1p if lay == 1 else We2p
            xr_nm = xr1_nm if lay == 1 else xr2_nm
            bB = bB1 if lay == 1 else bB2
            sfx = str(lay)
            Ls = meta["L" + sfx]
            seg, offs, nmax = Ls["seg"], Ls["offs"], Ls["nmax"]
            L = int(Ls["L"]); C = int(Ls["C"])
            if lay == 1:
                tlo = I["na_bf"][0:HALF, :]
                thi = I["na_bf"][HALF:NPAD1, :]
            else:
                tlo = tbl2[0:HALF, :]
                thi = tbl2[HALF:NPAD2, :]

            idx_all = pre.tile([128, L // 16], dt.int16, tag="idx" + sfx)
            nc.sync.dma_start(idx_all[:], I["idxs" + sfx][:])
            dlc_all = pre.tile([128, C], dt.float32, tag="dlc" + sfx)
            nc.scalar.dma_start(dlc_all[:], I["dlc" + sfx][:])

            # --- two-stage software-pipelined block loop: finalize stage A
            # (normalize + relu) runs one block behind the edge work, stage B
            # (transposes + layer-2 transforms / pooling) two blocks behind,
            # so the long cross-engine chains never stall the next block's
            # tensor stream.
            def fin_A(st):
                b = st["b"]
                if lay == 1:
                    aggf = ps_misc.tile([128, H1], dt.float32, tag="misc")
                    nc.tensor.matmul(aggf[:], st["U_sb"][:], Wl1p[:],
                                     start=True, stop=True)
                    aggp = aggf
                else:
                    aggp = st["agg"]
                t1 = sb.tile([128, H], dt.float32, tag="t1")
                nc.vector.tensor_scalar(t1[:], aggp[:, 0:H], st["rden"][:], None,
                                        op0=Alu.mult)
                t2 = sb.tile([128, H], dt.float32, tag="t2")
                nc.vector.tensor_tensor(t2[:], t1[:], bB[:], op=Alu.add)
                if DEBUG and lay == 1:
                    nc.sync.dma_start(dbg["t2"][b], t2[:])
                x_nm = sb.tile([128, H], dt.bfloat16, tag="xnm")
                nc.scalar.activation(x_nm[:], t2[:], AF.Relu)
                st["x_nm"] = x_nm

            def fin_B(st):
                b = st["b"]
                x_nm = st["x_nm"]
                if lay == 1:
                    for hh in range(H1 // 128):
                        tp = ps_tr.tile([128, 128], dt.bfloat16, tag="tr")
                        nc.tensor.transpose(tp[:], x_nm[:, hh * 128:(hh + 1) * 128],
                                            IDENT[:])
                        nc.scalar.copy(x1_T[:, hh, b * 128:(b + 1) * 128], tp[:])
                    psl = ps_misc.tile([128, H2], dt.float32, tag="misc")
                    for hh in range(H1 // 128):
                        nc.tensor.matmul(psl[:], x1_T[:, hh, b * 128:(b + 1) * 128],
                                         Wl2pp[:, hh * H2:(hh + 1) * H2],
                                         start=(hh == 0), stop=(hh == H1 // 128 - 1))
                    sbx = sb.tile([128, H2], dt.bfloat16, tag="sbx")
                    nc.vector.tensor_copy(sbx[:], psl[:])
                    nc.sync.dma_start(ag2_in[b * 128:(b + 1) * 128, :], sbx[:])
                    psr = ps_misc.tile([128, H2], dt.float32, tag="misc")
                    for hh in range(H1 // 128):
                        nc.tensor.matmul(psr[:], x1_T[:, hh, b * 128:(b + 1) * 128],
                                         Wr2pp[:, hh * H2:(hh + 1) * H2],
                                         start=(hh == 0), stop=(hh == H1 // 128 - 1))
                    nc.vector.tensor_tensor(xr2_nm[:, b, :], psr[:], brB2[:],
                                            op=Alu.add)
                else:
                    nc.tensor.matmul(pool_ps[:, 0:H2], PT_sb[b][:], x_nm[:],
                                     start=(b == 0), stop=(b == NBK - 1))

            pend_A = None
            pend_B = None
            for b in range(NBK):
                if lay == 1:
                    U_T = ps_U.tile([128, 128], dt.float32, tag="UT")
                    den = ps_den.tile([128, 8], dt.float32, tag="den")
                    agg = None
                else:
                    agg = ps_U.tile([128, H2 + 8], dt.float32, tag="agg2")
                    xlw = sbg.tile([128, H2], dt.bfloat16, tag="xlw")
                    nc.sync.dma_start(xlw[:], tbl2[ds(pid * BPC + b * 128, 128), :])
                # -- self-loop scores up-front (consts only; diag is ready
                # long before the block-tail aggregation needs it)
                s_s = ps_s4.tile([128, 2, H], dt.float32, tag="s4")
                if lay == 1:
                    nc.tensor.matmul(s_s[:, 0, :], natT[:, b, :], Wl1p[:],
                                     start=True, stop=False)
                else:
                    nc.tensor.matmul(s_s[:, 0, :], IDENT[:], xlw[:],
                                     start=True, stop=False)
                nc.tensor.matmul(s_s[:, 0, :], IDENT[:], xr_nm[:, b, :],
                                 start=False, stop=False)
                nc.tensor.matmul(s_s[:, 0, :], laT[:, b, :], We[:],
                                 start=False, stop=True)
                ls_s = sb.tile([128, 2, H], dt.bfloat16, tag="ls4")
                if Ppos > 0:
                    nc.scalar.activation(ls_s[:, 0, 0:Ppos], s_s[:, 0, 0:Ppos],
                                         AF.Prelu, alpha=0.2)
                if Ppos < H:
                    nc.scalar.activation(ls_s[:, 0, Ppos:H], s_s[:, 0, Ppos:H],
                                         AF.Prelu, scale=-0.2, alpha=5.0)
                es = sb.tile([128, 1], dt.float32, tag="es")
                nc.vector.reduce_sum(es[:], ls_s[:, 0:1, :], axis=mybir.AxisListType.X)
                ws = sb.tile([128, 1], dt.float32, tag="ws")
                nc.scalar.activation(ws[:], es[:], AF.Exp)
                if DEBUG and lay == 1:
                    nc.sync.dma_start(dbg["es"][b], es[:])
                diag = sb.tile([128, 128], dt.bfloat16, tag="diag")
                nc.vector.tensor_scalar(diag[:], IOTAF4[:, 0:128], iota_col[:],
                                        ws[:], op0=Alu.is_equal, op1=Alu.mult)

                first = True
                pend_g = None   # one-group-delayed MwT build + aggregation

                def flush_group():
                    nonlocal first, pend_g
                    if pend_g is None:
                        return
                    xlg_, po_, nch_, jj0_, w4_ = pend_g
                    for j in range(nch_):
                        cs = xlg_[:, po_ // 128 + j, :]
                        MwT = sb.tile([128, 128], dt.bfloat16, tag="mwt")
                        nc.vector.tensor_scalar(
                            MwT[:], IOTAF4[:, j * 128:(j + 1) * 128],
                            dlc_all[:, jj0_ + j:jj0_ + j + 1], w4_[:, j:j + 1],
                            op0=Alu.is_equal, op1=Alu.mult)
                        if lay == 1:
                            nc.tensor.matmul(U_T[:], cs, MwT[:],
                                             start=first, stop=False)
                            nc.tensor.matmul(den[:, 0:1], MwT[:], ones_col[:],
                                             start=first, stop=False)
                        else:
                            nc.tensor.matmul(agg[:, 0:H2], MwT[:], cs,
                                             start=first, stop=False)
                            nc.tensor.matmul(agg[:, H2:H2 + 1], MwT[:], ones_col[:],
                                             start=False, stop=False)
                        first = False
                    pend_g = None

                for h in range(2):
                    sl = int(seg[b, h]); o = int(offs[b, h])
                    if sl == 0:
                        continue
                    nch_all = sl // 128
                    xlg = sbg.tile([128, nch_all, 128], dt.bfloat16, tag="xlg")
                    nm = int(nmax[b, h])
                    if nm < sl:
                        # slots [nm:sl) are skipped by the gather (trailing
                        # negative idxs) and would hold stale SBUF data; zero
                        # the last chunk first so downstream exp/matmuls see
                        # finite values (the gather overwrites real rows).
                        nc.vector.memset(xlg[:, nch_all - 1, :], 0.0)
                    nc.gpsimd.dma_gather(xlg[:], thi if h else tlo,
                                         idx_all[:, o // 16:(o + sl) // 16],
                                         sl, nm, 128)
                    eTs = sb.tile([32, CHMX * 128], dt.bfloat16, tag="eT")
                    nc.sync.dma_start(eTs[:, :sl], I["eT" + sfx][:, o:o + sl])
                    M8s = sb.tile([128, CHMX * 128], dt.float8e4, tag="M8")
                    nc.scalar.dma_start(M8s[:, :sl], I["M8" + sfx][:, o:o + sl])
                    nagTs = []
                    if lay == 1:
                        # transpose all chunks up-front so the per-group score
                        # matmuls never wait on a fresh transpose+copy pair
                        for j in range(nch_all):
                            tp = ps_tr.tile([128, 128], dt.bfloat16, tag="tr")
                            nc.tensor.transpose(tp[:], xlg[:, j, :], IDENT[:])
                            nagT = nag.tile([128, 128], dt.bfloat16, tag="nagT")
                            nc.vector.tensor_copy(nagT[:], tp[:])
                            nagTs.append(nagT)
                    for po in range(0, sl, 256):
                        pl = min(256, sl - po)
                        nch = pl // 128
                        jj0 = (o + po) // 128
                        s4 = ps_s4.tile([128, 2, H], dt.float32, tag="s4")
                        for j in range(nch):
                            cs = xlg[:, po // 128 + j, :]
                            ec = slice(po + j * 128, po + (j + 1) * 128)
                            if lay == 1:
                                nc.tensor.matmul(s4[:, j, :], nagTs[po // 128 + j][:],
                                                 Wl1p[:], start=(j == 0), stop=False)
                            else:
                                nc.tensor.matmul(s4[:, j, :], IDENT[:], cs,
                                                 start=(j == 0), stop=False)
                            nc.tensor.matmul(s4[:, j, :], M8s[:, ec], xr_nm[:, b, :],
                                             start=False, stop=False)
                            nc.tensor.matmul(s4[:, j, :], eTs[:, ec], We[:],
                                             start=False, stop=(j == nch - 1))
                        ls4 = sb.tile([128, 2, H], dt.bfloat16, tag="ls4")
                        if Ppos > 0:
                            nc.scalar.activation(ls4[:, :nch, 0:Ppos], s4[:, :nch, 0:Ppos],
                                                 AF.Prelu, alpha=0.2)
                        if Ppos < H:
                            nc.scalar.activation(ls4[:, :nch, Ppos:H], s4[:, :nch, Ppos:H],
                                                 AF.Prelu, scale=-0.2, alpha=5.0)
                        e4 = sb.tile([128, 2], dt.float32, tag="e4")
                        nc.vector.reduce_sum(e4[:, :nch], ls4[:, :nch, :],
                                             axis=mybir.AxisListType.X)
                        w4 = sb.tile([128, 2], dt.float32, tag="w4")
                        nc.scalar.activation(w4[:, :nch], e4[:, :nch], AF.Exp)
                        if DEBUG and lay == 1 and h == 0 and po == 0:
                            nc.sync.dma_start(dbg["e4"][b], e4[:])
                        flush_group()
                        pend_g = (xlg, po, nch, jj0, w4)
                flush_group()
                # -- self-loop aggregation (diag ready since block start)
                if lay == 1:
                    nc.tensor.matmul(U_T[:], na_own[:, b, :], diag[:],
                                     start=False, stop=True)
                    nc.tensor.matmul(den[:, 0:1], diag[:], ones_col[:],
                                     start=False, stop=True)
                else:
                    nc.tensor.matmul(agg[:, 0:H2], diag[:], xlw[:],
                                     start=False, stop=False)
                    nc.tensor.matmul(agg[:, H2:H2 + 1], diag[:], ones_col[:],
                                     start=False, stop=True)
                # -- early epilog: free U_T/den for the next block right away
                st = dict(b=b, agg=agg)
                rden = sb.tile([128, 1], dt.float32, tag="rden")
                if lay == 1:
                    U_sb = sb.tile([128, 128], dt.bfloat16, tag="usb")
                    nc.vector.tensor_copy(U_sb[:], U_T[:])
                    nc.vector.reciprocal(rden[:], den[:, 0:1])
                    st["U_sb"] = U_sb
                    if DEBUG:
                        nc.sync.dma_start(dbg["usb"][b], U_sb[:])
                        nc.sync.dma_start(dbg["rden"][b], rden[:])
                else:
                    nc.vector.reciprocal(rden[:], agg[:, H2:H2 + 1])
                st["rden"] = rden
                # -- delayed finalize stages
                if pend_B is not None:
                    fin_B(pend_B)
                pend_B = None
                if pend_A is not None:
                    fin_A(pend_A)
                    pend_B = pend_A
                pend_A = st
            if pend_B is not None:
                fin_B(pend_B)
            if pend_A is not None:
                fin_A(pend_A)
                fin_B(pend_A)

        # layer-1 edge phase
        with ExitStack() as ctx1:
            ps_s4 = ctx1.enter_context(tc.tile_pool(name="ps_s4", bufs=2, space="PSUM"))
            ps_tr = ctx1.enter_context(tc.tile_pool(name="ps_tr", bufs=1, space="PSUM"))
            ps_U = ctx1.enter_context(tc.tile_pool(name="ps_U", bufs=2, space="PSUM"))
            ps_den = ctx1.enter_context(tc.tile_pool(name="ps_den", bufs=1, space="PSUM"))
            ps_misc = ctx1.enter_context(tc.tile_pool(name="ps_misc", bufs=2, space="PSUM"))
            edge_phase(1, ps_s4, ps_tr, ps_U, ps_den, ps_misc)

        nc.gpsimd.collective_compute(
            "AllGather", mybir.AluOpType.bypass,
            replica_groups=[list(range(NCORE))],
            ins=[ag2_in[:]], outs=[tbl2[:]])

        # ---------------- layer-2 edge phase + pooling ------------------
        pool_pp = ctx.enter_context(tc.tile_pool(name="poolps", bufs=1, space="PSUM"))
        pool_ps = pool_pp.tile([G, H2 + 4], dt.float32, tag="pool")
        pt_pool = ctx.enter_context(tc.tile_pool(name="ptsb", bufs=1))
        PT_sb = []
        for b in range(NBK):
            t = pt_pool.tile([128, G], dt.bfloat16, tag=f"pt{b}")
            nc.sync.dma_start(t[:], I["PT"][b])
            PT_sb.append(t)
        with ExitStack() as ctx2:
            ps_s4 = ctx2.enter_context(tc.tile_pool(name="ps_s4b", bufs=2, space="PSUM"))
            ps_tr = ctx2.enter_context(tc.tile_pool(name="ps_trb", bufs=1, space="PSUM"))
            ps_U = ctx2.enter_context(tc.tile_pool(name="ps_Ub", bufs=2, space="PSUM"))
            ps_den = ctx2.enter_context(tc.tile_pool(name="ps_denb", bufs=1, space="PSUM"))
            ps_misc = ctx2.enter_context(tc.tile_pool(name="ps_miscb", bufs=2, space="PSUM"))
            edge_phase(2, ps_s4, ps_tr, ps_U, ps_den, ps_misc,
                       pool_ps=pool_ps, PT_sb=PT_sb)

        # ---------------- head -----------------------------------------
        with tc.tile_pool(name="hsb", bufs=2) as hsb, \
             tc.tile_pool(name="hps", bufs=2, space="PSUM") as hps:
            psb = hsb.tile([G, H2], dt.float32, tag="poolsb")
            nc.scalar.copy(psb[:], pool_ps[:, 0:H2])
            nc.sync.dma_start(pool_in[:], psb[:])
            nc.gpsimd.collective_compute(
                "AllReduce", mybir.AluOpType.add,
                replica_groups=[list(range(NCORE))],
                ins=[pool_in[:]], outs=[pool_out[:]])
            pooled = hsb.tile([G, H2], dt.float32, tag="pooled")
            nc.sync.dma_start(pooled[:], pool_out[:])
            pooled_T_ps = hps.tile([H2, G], dt.float32, tag="pooledT")
            nc.tensor.transpose(pooled_T_ps[:], pooled[:], IDENT32[0:G, 0:G])
            pooled_T = hsb.tile([H2, G], dt.float32, tag="pooledTsb")
            nc.scalar.copy(pooled_T[:], pooled_T_ps[:])
            Wd1sb = hsb.tile([H2, HD], dt.float32, tag="wd1")
            nc.sync.dma_start(Wd1sb[:], I["Wd1u"][:])
            h1ps = hps.tile([HD, G], dt.float32, tag="h1")
            nc.tensor.matmul(h1ps[:], Wd1sb[:], pooled_T[:], start=True, stop=True)
            hscale = hsb.tile([HD, 1], dt.float32, tag="hscale")
            nc.sync.dma_start(hscale[:], I["head_scale"][:])
            hbias = hsb.tile([HD, 1], dt.float32, tag="hbias")
            nc.sync.dma_start(hbias[:], I["head_bias"][:])
            th = hsb.tile([HD, G], dt.float32, tag="th")
            nc.scalar.activation(th[:], h1ps[:], AF.Prelu, bias=hbias[:],
                                 scale=hscale[:], alpha=0.1)
            Wd2sb = hsb.tile([HD, OUT], dt.float32, tag="wd2")
            nc.sync.dma_start(Wd2sb[:], I["Wd2"][:])
            ops = hps.tile([OUT, G], dt.float32, tag="ops")
            nc.tensor.matmul(ops[:], Wd2sb[:], th[:], start=True, stop=True)
            bd2sb = hsb.tile([OUT, 1], dt.float32, tag="bd2sb")
            nc.sync.dma_start(bd2sb[:], I["bd2"][:])
            osb = hsb.tile([OUT, G], dt.float32, tag="osb")
            nc.vector.tensor_scalar(osb[:], ops[:], bd2sb[:], None, op0=Alu.add)
            nc.sync.dma_start(out_t[:], osb[:])


def _kernel(inputs, cfg, runner=None, trace=False):
    com, percore, meta = host_prep(inputs, cfg)
    nc = build_program(meta, com, percore[0])
    in_maps = [dict(com, **pc) for pc in percore]
    if runner is None:
        from concourse.bass_utils import run_bass_kernel_spmd
        res = run_bass_kernel_spmd(nc, in_maps, list(range(cfg["NC"])), trace=trace)
        out = np.asarray(res.results[0]["out"])
        return out.T.copy().astype(np.float32), res
    return runner(nc, in_maps)


def kernel(**inputs):
    out, _ = _kernel(inputs, DEFAULT_CFG)
    return out


# revision 36
# speedup vs baseline: 1.2622x; 1.0457x over previous
"""GATv2 x2 + global-mean-pool + MLP head on 8 NeuronCores (Bass/Tile).

Sharding: destination-partitioned. Core c owns nodes [c*NPC, (c+1)*NPC);
it processes every edge whose dst is in its range, so attention softmax
segments are core-local.

Layer 1 gathers RAW node_attr rows (256B) per edge — no xl1 table is
ever materialized.  The per-chunk score xl-term is a matmul of the
transposed gathered rows with Wl1; the aggregation accumulates
U_T[DIN, d] += NA_g.T @ MwT per chunk and applies Wl1 once per block.
Layer 2 AllGathers the raw xl2 table (x1 @ Wl2, no bias) and gathers
its 256B rows per edge.

Host precomputes: loop_attr (self-loop edge features), per-chunk dst
one-hot matrices M [dstrow, edge] (streamed, fp8), dst-local-row
columns, and folds |att| into the weights (channels permuted so
positive-att channels come first; see baseline notes).  All biases are
folded: the score-side bias (bl+br) rides on xr; the output-side bias
(bl+b) is added at block finalize (valid since softmax weights sum to
1).  exp is applied without max-subtraction: logits are O(1) here.
"""

import sys
import numpy as np
import ml_dtypes

sys.path.insert(0, "/opt/trn_rl_repo")

BF16 = ml_dtypes.bfloat16
F8 = ml_dtypes.float8_e4m3

DEFAULT_CFG = dict(
    N=50000, E=500000, G=64,
    DIN=128, ED=32, H1=256, H2=128, HD=64, OUT=8,
    NC=8, HALF=32768,
)


def _roundup(x, m):
    return (x + m - 1) // m * m


def _wrap16(idx, L):
    out = np.full((128, max(L // 16, 1)), -1, np.int16)
    n = len(idx)
    if n:
        pos = np.arange(n)
        out[pos % 16, pos // 16] = idx.astype(np.int16)
    for g in range(1, 8):
        out[g * 16:(g + 1) * 16] = out[0:16]
    return out


def host_prep(inputs, cfg):
    c = dict(cfg)
    N, E, G = c["N"], c["E"], c["G"]
    DIN, ED, H1, H2 = c["DIN"], c["ED"], c["H1"], c["H2"]
    NCORE, HALF = c["NC"], c["HALF"]
    NPC = N // NCORE
    NBK = _roundup(NPC, 128) // 128
    BPC = NBK * 128
    NPAD1 = _roundup(N, 512)
    NPAD2 = NCORE * BPC

    f64 = lambda x: np.asarray(x, np.float64)
    att1, att2 = f64(inputs["att1"]), f64(inputs["att2"])
    a1 = np.maximum(np.abs(att1), 1e-12); s1 = np.where(att1 >= 0, 1.0, -1.0)
    a2 = np.maximum(np.abs(att2), 1e-12); s2 = np.where(att2 >= 0, 1.0, -1.0)
    perm1 = np.argsort(-s1, kind="stable"); P1 = int((s1 > 0).sum())
    perm2 = np.argsort(-s2, kind="stable"); P2 = int((s2 > 0).sum())
    a1p, a2p = a1[perm1], a2[perm2]

    Wl1p = (f64(inputs["Wl1"]) * a1)[:, perm1]
    Wr1p = (f64(inputs["Wr1"]) * a1)[:, perm1]
    We1p = (f64(inputs["We1"]) * a1)[:, perm1]
    bl1p = (f64(inputs["bl1"]) * a1)[perm1]
    br1p = (f64(inputs["br1"]) * a1)[perm1]
    b1p = (f64(inputs["b1"]) * a1)[perm1]

    Wl2u = f64(inputs["Wl2"])[perm1, :] / a1p[:, None]
    Wr2u = f64(inputs["Wr2"])[perm1, :] / a1p[:, None]
    Wl2pp = (Wl2u * a2)[:, perm2]
    Wr2pp = (Wr2u * a2)[:, perm2]
    We2p = (f64(inputs["We2"]) * a2)[:, perm2]
    bl2p = (f64(inputs["bl2"]) * a2)[perm2]
    br2p = (f64(inputs["br2"]) * a2)[perm2]
    b2p = (f64(inputs["b2"]) * a2)[perm2]

    Wd1u = f64(inputs["Wd1"])[perm2, :] / a2p[:, None]
    bs = f64(inputs["bn_gamma"]) / np.sqrt(f64(inputs["bn_var"]) + 1e-5)
    head_scale = bs
    head_bias = (f64(inputs["bd1"]) * bs + f64(inputs["bn_beta"])
                 - f64(inputs["bn_mean"]) * bs)

    src = np.asarray(inputs["edge_src"], np.int64)
    dst = np.asarray(inputs["edge_dst"], np.int64)
    batch = np.asarray(inputs["batch"], np.int64)
    eattr = np.asarray(inputs["edge_attr"], np.float64)

    # loop_attr (self-loop edge features) on host: segment mean of eattr by dst
    deg = np.bincount(dst, minlength=N).astype(np.float64)
    order_d = np.argsort(dst, kind="stable")
    eattr_sorted = eattr[order_d]
    cuts = np.searchsorted(dst[order_d], np.arange(N))
    la = np.zeros((N, ED), np.float64)
    nz = deg > 0
    sums = np.add.reduceat(eattr_sorted, np.minimum(cuts, len(dst) - 1), axis=0)
    la[nz] = sums[nz] / deg[nz][:, None]

    core_of = dst // NPC
    blk_of = (dst % NPC) // 128
    dloc_of = (dst % NPC) % 128

    def layer_streams(row):
        half = (row >= HALF).astype(np.int64)
        cnt = np.zeros((NCORE, NBK, 2), np.int64)
        np.add.at(cnt, (core_of, blk_of, half), 1)
        nmax = cnt.max(axis=0)                         # [NBK, 2] real rows
        nmax[:, 0] = np.maximum(nmax[:, 0], 1)
        seg = _roundup(nmax, 128)                      # [NBK, 2]
        seg[:, 0] = np.maximum(seg[:, 0], 128)
        offs = np.zeros((NBK, 2), np.int64)
        L = 0
        for b in range(NBK):
            for h in range(2):
                offs[b, h] = L
                L += seg[b, h]
        C = L // 128
        key = core_of * (NBK * 2) + blk_of * 2 + half
        order = np.argsort(key, kind="stable")
        ks = key[order]
        idxs = np.zeros((NCORE, 128, L // 16), np.int16)
        eT = np.zeros((NCORE, ED, L), BF16)
        M8 = np.zeros((NCORE, 128, L), F8)
        MT = np.zeros((NCORE, 128, L), BF16)
        bounds = np.searchsorted(ks, np.arange(NCORE * NBK * 2 + 1))
        for cr in range(NCORE):
            for b in range(NBK):
                for h in range(2):
                    k = cr * (NBK * 2) + b * 2 + h
                    m = order[bounds[k]:bounds[k + 1]]
                    n = len(m)
                    o = int(offs[b, h]); sl = int(seg[b, h])
                    if sl == 0:
                        continue
                    nm = int(nmax[b, h])
                    loc_idx = np.full(sl, -1, np.int64)
                    loc_idx[:nm] = 0
                    loc_idx[:n] = row[m] - h * HALF
                    idxs[cr][:, o // 16:(o + sl) // 16] = _wrap16(loc_idx, sl)
                    if n:
                        eT[cr][:, o:o + n] = eattr[m].T.astype(BF16)
                        M8[cr][dloc_of[m], o + np.arange(n)] = F8(1.0)
                        p = np.arange(n)
                        # MT chunk c is the [e, d] one-hot: row e%128 of chunk
                        # (o+e)//128 has a 1 at free-col dloc
                        MT[cr][p % 128, (o + p) // 128 * 128 + dloc_of[m]] = BF16(1.0)
        return dict(seg=seg, offs=offs, nmax=nmax, L=L, C=C, idxs=idxs, eT=eT,
                    M8=M8, MT=MT)

    row1 = src
    row2 = BPC * (src // NPC) + (src % NPC)
    L1s = layer_streams(row1)
    L2s = layer_streams(row2)
    CHMX = int(max(L1s["seg"].max(), L2s["seg"].max())) // 128

    cnts = np.maximum(np.bincount(batch, minlength=G).astype(np.float64), 1.0)
    PT = np.zeros((NCORE, NBK, 128, G), BF16)
    for cr in range(NCORE):
        for b in range(NBK):
            base = cr * NPC + b * 128
            nn = min(128, NPC - b * 128)
            if nn <= 0:
                continue
            gids = batch[base:base + nn]
            PT[cr, b, np.arange(nn), gids] = (1.0 / cnts[gids]).astype(BF16)

    IDENT = np.eye(128, dtype=BF16)
    IDENT32 = np.eye(128, dtype=np.float32)
    ones_col = np.ones((128, 1), BF16)

    na_bf = np.zeros((NPAD1, DIN), BF16)
    na_f32 = np.asarray(inputs["node_attr"], np.float32)
    na_bf[:N] = na_f32.astype(BF16)

    # per-core own-node views (self-loop chunks + xr transform)
    natT = np.zeros((NCORE, DIN, NBK, 128), BF16)
    na_own = np.zeros((NCORE, 128, NBK, DIN), BF16)
    laT = np.zeros((NCORE, ED, NBK, 128), BF16)
    for cr in range(NCORE):
        for b in range(NBK):
            base = cr * NPC + b * 128
            nn = min(128, N - base) if base < N else 0
            nn = min(nn, NPC - b * 128)
            if nn <= 0:
                continue
            natT[cr, :, b, :nn] = na_f32[base:base + nn].T.astype(BF16)
            na_own[cr, :nn, b, :] = na_f32[base:base + nn].astype(BF16)
            laT[cr, :, b, :nn] = la[base:base + nn].T.astype(BF16)

    bcast = lambda v: np.tile(np.asarray(v, np.float32)[None, :], (128, 1)).copy()

    com = dict(
        na_bf=na_bf,
        Wl1p=Wl1p.astype(BF16), Wr1p=Wr1p.astype(BF16), We1p=We1p.astype(BF16),
        Wl2pp=Wl2pp.reshape(H1 // 128, 128, H2).transpose(1, 0, 2).reshape(128, -1).astype(BF16),
        Wr2pp=Wr2pp.reshape(H1 // 128, 128, H2).transpose(1, 0, 2).reshape(128, -1).astype(BF16),
        We2p=We2p.astype(BF16),
        brB1=bcast(bl1p + br1p), bB1=bcast(bl1p + b1p),
        brB2=bcast(bl2p + br2p), bB2=bcast(bl2p + b2p),
        Wd1u=Wd1u.astype(np.float32),
        head_scale=head_scale.astype(np.float32).reshape(-1, 1),
        head_bias=head_bias.astype(np.float32).reshape(-1, 1),
        Wd2=np.asarray(inputs["Wd2"], np.float32),
        bd2=np.asarray(inputs["bd2"], np.float32).reshape(-1, 1),
        IDENT=IDENT, IDENT32=IDENT32,
        ones_col=ones_col,
    )
    percore = []
    for cr in range(NCORE):
        percore.append(dict(
            idxs1=L1s["idxs"][cr], eT1=L1s["eT"][cr], M81=L1s["M8"][cr],
            MT1=L1s["MT"][cr],
            idxs2=L2s["idxs"][cr], eT2=L2s["eT"][cr], M82=L2s["M8"][cr],
            MT2=L2s["MT"][cr],
            PT=PT[cr], natT=natT[cr], na_own=na_own[cr], laT=laT[cr],
        ))
    meta = dict(cfg=c, NPC=NPC, NBK=NBK, BPC=BPC, NPAD1=NPAD1, NPAD2=NPAD2,
                P1=P1, P2=P2, L1=L1s, L2=L2s, CHMX=CHMX)
    return com, percore, meta


def build_program(meta, com, pc0):
    import concourse.bass as bass
    import concourse.tile as tile
    from concourse import bacc, mybir
    from concourse import library_config

    c = meta["cfg"]
    G, H2, OUT = c["G"], c["H2"], c["OUT"]
    NCORE = c["NC"]
    BPC = meta["BPC"]
    NPAD2 = meta["NPAD2"]
    dt = mybir.dt

    nc = bacc.Bacc("TRN2", target_bir_lowering=False, debug=False,
                   num_devices=NCORE)

    dmap = {np.dtype(np.float32): dt.float32, np.dtype(BF16): dt.bfloat16,
            np.dtype(np.int16): dt.int16, np.dtype(F8): dt.float8e4}
    I = {}
    for d in (com, pc0):
        for k, a in d.items():
            I[k] = nc.dram_tensor(k, list(a.shape), dmap[a.dtype],
                                  kind="ExternalInput")

    out_t = nc.dram_tensor("out", [OUT, G], dt.float32, kind="ExternalOutput")
    NBK = meta["NBK"]
    H1 = c["H1"]
    dbg = dict(
        rden=nc.dram_tensor("dbg_rden", [NBK, 128, 1], dt.float32),
        es=nc.dram_tensor("dbg_es", [NBK, 128, 1], dt.float32),
        usb=nc.dram_tensor("dbg_usb", [NBK, 128, 128], dt.bfloat16),
        t2=nc.dram_tensor("dbg_t2", [NBK, 128, H1], dt.float32),
        e4=nc.dram_tensor("dbg_e4", [NBK, 128, 2], dt.float32),
    )
    ag2_in = nc.dram_tensor("ag2_in", [BPC, H2], dt.bfloat16)
    tbl2 = nc.dram_tensor("tbl2", [NPAD2, H2], dt.bfloat16, addr_space="Shared")
    pool_in = nc.dram_tensor("pool_in", [G, H2], dt.float32)
    pool_out = nc.dram_tensor("pool_out", [G, H2], dt.float32, addr_space="Shared")

    with tile.TileContext(nc) as tc:
        _body(nc, tc, I, out_t, ag2_in, tbl2, pool_in, pool_out,
              meta, bass, tile, mybir, library_config, dbg=dbg)
    nc.compile()
    return nc


DEBUG = False


def _body(nc, tc, I, out_t, ag2_in, tbl2, pool_in, pool_out,
          meta, bass, tile, mybir, library_config, dbg=None):
    from contextlib import ExitStack

    c = meta["cfg"]
    G = c["G"]
    DIN, ED, H1, H2, HD, OUT = c["DIN"], c["ED"], c["H1"], c["H2"], c["HD"], c["OUT"]
    NCORE, HALF = c["NC"], c["HALF"]
    NPC, NBK, BPC = meta["NPC"], meta["NBK"], meta["BPC"]
    NPAD1, NPAD2 = meta["NPAD1"], meta["NPAD2"]
    P1, P2 = meta["P1"], meta["P2"]
    CHMX = meta["CHMX"]
    AF = mybir.ActivationFunctionType
    dt = mybir.dt
    Alu = mybir.AluOpType
    ds = bass.ds

    nc.gpsimd.load_library(library_config.mlp)
    pid = nc.partition_id()

    ctx = ExitStack()
    with ctx:
        consts = ctx.enter_context(tc.tile_pool(name="consts", bufs=1))

        def cload(name):
            a = I[name]
            t = consts.tile(list(a.shape), a.dtype, tag=name)
            nc.sync.dma_start(t[:], a[:])
            return t

        IDENT = cload("IDENT")
        IDENT32 = cload("IDENT32")
        ones_col = cload("ones_col")
        Wl1p = cload("Wl1p"); Wr1p = cload("Wr1p"); We1p = cload("We1p")
        Wl2pp = cload("Wl2pp"); Wr2pp = cload("Wr2pp"); We2p = cload("We2p")
        brB1 = cload("brB1"); bB1 = cload("bB1")
        brB2 = cload("brB2"); bB2 = cload("bB2")
        natT = cload("natT"); na_own = cload("na_own"); laT = cload("laT")

        res = ctx.enter_context(tc.tile_pool(name="res", bufs=1))
        xr1_nm = res.tile([128, NBK, H1], dt.bfloat16, tag="xr1")
        x1_T = res.tile([128, H1 // 128, BPC], dt.bfloat16, tag="x1T")
        xr2_nm = res.tile([128, NBK, H2], dt.bfloat16, tag="xr2")

        # ---------------- phase 0: xr1 for own nodes -------------------
        with tc.tile_pool(name="p0ps", bufs=2, space="PSUM") as p0ps:
            for b in range(NBK):
                ps = p0ps.tile([128, H1], dt.float32, tag="xr1ps")
                nc.tensor.matmul(ps[:], natT[:, b, :], Wr1p[:],
                                 start=True, stop=True)
                nc.vector.tensor_tensor(xr1_nm[:, b, :], ps[:], brB1[:], op=Alu.add)

        # ---------------- shared pools for both edge phases ------------
        sb = ctx.enter_context(tc.tile_pool(name="sb", bufs=4))
        sbg = ctx.enter_context(tc.tile_pool(name="sbg", bufs=6))
        nag = ctx.enter_context(tc.tile_pool(name="nag", bufs=CHMX + 4))
        pre = ctx.enter_context(tc.tile_pool(name="pre", bufs=1))



        # ---------------- shared edge phase ----------------------------
        def edge_phase(lay, ps_s4, ps_tr, ps_U, ps_den, ps_misc,
                       pool_ps=None, PT_sb=None):
            H = H1 if lay == 1 else H2
            Ppos = P1 if lay == 1 else P2
            We = We1p if lay == 1 else We2p
            xr_nm = xr1_nm if lay == 1 else xr2_nm
            bB = bB1 if lay == 1 else bB2
            sfx = str(lay)
            Ls = meta["L" + sfx]
            seg, offs, nmax = Ls["seg"], Ls["offs"], Ls["nmax"]
            L = int(Ls["L"]); C = int(Ls["C"])
            if lay == 1:
                tlo = I["na_bf"][0:HALF, :]
                thi = I["na_bf"][HALF:NPAD1, :]
            else:
                tlo = tbl2[0:HALF, :]
                thi = tbl2[HALF:NPAD2, :]

            idx_all = pre.tile([128, L // 16], dt.int16, tag="idx" + sfx)
            nc.sync.dma_start(idx_all[:], I["idxs" + sfx][:])

            # --- two-stage software-pipelined block loop: finalize stage A
            # (normalize + relu) runs one block behind the edge work, stage B
            # (transposes + layer-2 transforms / pooling) two blocks behind,
            # so the long cross-engine chains never stall the next block's
            # tensor stream.
            def fin_A(st):
                b = st["b"]
                if lay == 1:
                    aggf = ps_misc.tile([128, H1], dt.float32, tag="misc")
                    nc.tensor.matmul(aggf[:], st["U_sb"][:], Wl1p[:],
                                     start=True, stop=True)
                    aggp = aggf
                else:
                    aggp = st["agg"]
                t1 = sb.tile([128, H], dt.float32, tag="t1")
                nc.vector.tensor_scalar(t1[:], aggp[:, 0:H], st["rden"][:], None,
                                        op0=Alu.mult)
                t2 = sb.tile([128, H], dt.float32, tag="t2")
                nc.vector.tensor_tensor(t2[:], t1[:], bB[:], op=Alu.add)
                if DEBUG and lay == 1:
                    nc.sync.dma_start(dbg["t2"][b], t2[:])
                x_nm = sb.tile([128, H], dt.bfloat16, tag="xnm")
                nc.scalar.activation(x_nm[:], t2[:], AF.Relu)
                st["x_nm"] = x_nm

            def fin_B(st):
                b = st["b"]
                x_nm = st["x_nm"]
                if lay == 1:
                    for hh in range(H1 // 128):
                        tp = ps_tr.tile([128, 128], dt.bfloat16, tag="tr")
                        nc.tensor.transpose(tp[:], x_nm[:, hh * 128:(hh + 1) * 128],
                                            IDENT[:])
                        nc.scalar.copy(x1_T[:, hh, b * 128:(b + 1) * 128], tp[:])
                    psl = ps_misc.tile([128, H2], dt.float32, tag="misc")
                    for hh in range(H1 // 128):
                        nc.tensor.matmul(psl[:], x1_T[:, hh, b * 128:(b + 1) * 128],
                                         Wl2pp[:, hh * H2:(hh + 1) * H2],
                                         start=(hh == 0), stop=(hh == H1 // 128 - 1))
                    sbx = sb.tile([128, H2], dt.bfloat16, tag="sbx")
                    nc.vector.tensor_copy(sbx[:], psl[:])
                    nc.sync.dma_start(ag2_in[b * 128:(b + 1) * 128, :], sbx[:])
                    psr = ps_misc.tile([128, H2], dt.float32, tag="misc")
                    for hh in range(H1 // 128):
                        nc.tensor.matmul(psr[:], x1_T[:, hh, b * 128:(b + 1) * 128],
                                         Wr2pp[:, hh * H2:(hh + 1) * H2],
                                         start=(hh == 0), stop=(hh == H1 // 128 - 1))
                    nc.vector.tensor_tensor(xr2_nm[:, b, :], psr[:], brB2[:],
                                            op=Alu.add)
                else:
                    nc.tensor.matmul(pool_ps[:, 0:H2], PT_sb[b][:], x_nm[:],
                                     start=(b == 0), stop=(b == NBK - 1))

            pend_A = None
            pend_B = None
            for b in range(NBK):
                if lay == 1:
                    U_T = ps_U.tile([128, 128], dt.float32, tag="UT")
                    den = ps_den.tile([128, 8], dt.float32, tag="den")
                    agg = None
                else:
                    agg = ps_U.tile([128, H2 + 8], dt.float32, tag="agg2")
                    xlw = sbg.tile([128, H2], dt.bfloat16, tag="xlw")
                    nc.sync.dma_start(xlw[:], tbl2[ds(pid * BPC + b * 128, 128), :])
                # -- self-loop scores up-front (consts only; diag is ready
                # long before the block-tail aggregation needs it)
                s_s = ps_s4.tile([128, 2, H], dt.float32, tag="s4")
                if lay == 1:
                    nc.tensor.matmul(s_s[:, 0, :], natT[:, b, :], Wl1p[:],
                                     start=True, stop=False)
                else:
                    nc.tensor.matmul(s_s[:, 0, :], IDENT[:], xlw[:],
                                     start=True, stop=False)
                nc.tensor.matmul(s_s[:, 0, :], IDENT[:], xr_nm[:, b, :],
                                 start=False, stop=False)
                nc.tensor.matmul(s_s[:, 0, :], laT[:, b, :], We[:],
                                 start=False, stop=True)
                ls_s = sb.tile([128, 2, H], dt.bfloat16, tag="ls4")
                if Ppos > 0:
                    nc.scalar.activation(ls_s[:, 0, 0:Ppos], s_s[:, 0, 0:Ppos],
                                         AF.Prelu, alpha=0.2)
                if Ppos < H:
                    nc.scalar.activation(ls_s[:, 0, Ppos:H], s_s[:, 0, Ppos:H],
                                         AF.Prelu, scale=-0.2, alpha=5.0)
                es = sb.tile([128, 1], dt.float32, tag="es")
                nc.vector.reduce_sum(es[:], ls_s[:, 0:1, :], axis=mybir.AxisListType.X)
                ws = sb.tile([128, 1], dt.float32, tag="ws")
                nc.scalar.activation(ws[:], es[:], AF.Exp)
                if DEBUG and lay == 1:
                    nc.sync.dma_start(dbg["es"][b], es[:])
                diag = sb.tile([128, 128], dt.bfloat16, tag="diag")
                nc.vector.tensor_scalar(diag[:], IDENT[:], ws[:], None, op0=Alu.mult)

                first = True
                pend_g = None   # one-group-delayed MwT build + aggregation

                def flush_group():
                    nonlocal first, pend_g
                    if pend_g is None:
                        return
                    xlg_, po_, nch_, MTs_, w4_ = pend_g
                    for j in range(nch_):
                        cs = xlg_[:, po_ // 128 + j, :]
                        ec = slice(po_ + j * 128, po_ + (j + 1) * 128)
                        MwT = sb.tile([128, 128], dt.bfloat16, tag="mwt")
                        nc.vector.tensor_scalar(
                            MwT[:], MTs_[:, ec], w4_[:, j:j + 1], None,
                            op0=Alu.mult)
                        if lay == 1:
                            nc.tensor.matmul(U_T[:], cs, MwT[:],
                                             start=first, stop=False)
                            nc.tensor.matmul(den[:, 0:1], MwT[:], ones_col[:],
                                             start=first, stop=False)
                        else:
                            nc.tensor.matmul(agg[:, 0:H2], MwT[:], cs,
                                             start=first, stop=False)
                            nc.tensor.matmul(agg[:, H2:H2 + 1], MwT[:], ones_col[:],
                                             start=False, stop=False)
                        first = False
                    pend_g = None

                for h in range(2):
                    sl = int(seg[b, h]); o = int(offs[b, h])
                    if sl == 0:
                        continue
                    nch_all = sl // 128
                    xlg = sbg.tile([128, nch_all, 128], dt.bfloat16, tag="xlg")
                    nm = int(nmax[b, h])
                    if nm < sl:
                        # slots [nm:sl) are skipped by the gather (trailing
                        # negative idxs) and would hold stale SBUF data; zero
                        # the last chunk first so downstream exp/matmuls see
                        # finite values (the gather overwrites real rows).
                        nc.vector.memset(xlg[:, nch_all - 1, :], 0.0)
                    nc.gpsimd.dma_gather(xlg[:], thi if h else tlo,
                                         idx_all[:, o // 16:(o + sl) // 16],
                                         sl, nm, 128)
                    eTs = sb.tile([32, CHMX * 128], dt.bfloat16, tag="eT")
                    nc.sync.dma_start(eTs[:, :sl], I["eT" + sfx][:, o:o + sl])
                    M8s = sb.tile([128, CHMX * 128], dt.float8e4, tag="M8")
                    nc.scalar.dma_start(M8s[:, :sl], I["M8" + sfx][:, o:o + sl])
                    MTs = sb.tile([128, CHMX * 128], dt.bfloat16, tag="MT")
                    nc.scalar.dma_start(MTs[:, :sl], I["MT" + sfx][:, o:o + sl])
                    nagTs = []
                    if lay == 1:
                        # transpose all chunks up-front so the per-group score
                        # matmuls never wait on a fresh transpose+copy pair
                        for j in range(nch_all):
                            tp = ps_tr.tile([128, 128], dt.bfloat16, tag="tr")
                            nc.tensor.transpose(tp[:], xlg[:, j, :], IDENT[:])
                            nagT = nag.tile([128, 128], dt.bfloat16, tag="nagT")
                            nc.vector.tensor_copy(nagT[:], tp[:])
                            nagTs.append(nagT)
                    for po in range(0, sl, 256):
                        pl = min(256, sl - po)
                        nch = pl // 128
                        jj0 = (o + po) // 128
                        s4 = ps_s4.tile([128, 2, H], dt.float32, tag="s4")
                        for j in range(nch):
                            cs = xlg[:, po // 128 + j, :]
                            ec = slice(po + j * 128, po + (j + 1) * 128)
                            if lay == 1:
                                nc.tensor.matmul(s4[:, j, :], nagTs[po // 128 + j][:],
                                                 Wl1p[:], start=(j == 0), stop=False)
                            else:
                                nc.tensor.matmul(s4[:, j, :], IDENT[:], cs,
                                                 start=(j == 0), stop=False)
                            nc.tensor.matmul(s4[:, j, :], M8s[:, ec], xr_nm[:, b, :],
                                             start=False, stop=False)
                            nc.tensor.matmul(s4[:, j, :], eTs[:, ec], We[:],
                                             start=False, stop=(j == nch - 1))
                        ls4 = sb.tile([128, 2, H], dt.bfloat16, tag="ls4")
                        if Ppos > 0:
                            nc.scalar.activation(ls4[:, :nch, 0:Ppos], s4[:, :nch, 0:Ppos],
                                                 AF.Prelu, alpha=0.2)
                        if Ppos < H:
                            nc.scalar.activation(ls4[:, :nch, Ppos:H], s4[:, :nch, Ppos:H],
                                                 AF.Prelu, scale=-0.2, alpha=5.0)
                        e4 = sb.tile([128, 2], dt.float32, tag="e4")
                        nc.vector.reduce_sum(e4[:, :nch], ls4[:, :nch, :],
                                             axis=mybir.AxisListType.X)
                        w4 = sb.tile([128, 2], dt.float32, tag="w4")
                        nc.scalar.activation(w4[:, :nch], e4[:, :nch], AF.Exp)
                        if DEBUG and lay == 1 and h == 0 and po == 0:
                            nc.sync.dma_start(dbg["e4"][b], e4[:])
                        flush_group()
                        pend_g = (xlg, po, nch, MTs, w4)
                flush_group()
                # -- self-loop aggregation (diag ready since block start)
                if lay == 1:
                    nc.tensor.matmul(U_T[:], na_own[:, b, :], diag[:],
                                     start=False, stop=True)
                    nc.tensor.matmul(den[:, 0:1], diag[:], ones_col[:],
                                     start=False, stop=True)
                else:
                    nc.tensor.matmul(agg[:, 0:H2], diag[:], xlw[:],
                                     start=False, stop=False)
                    nc.tensor.matmul(agg[:, H2:H2 + 1], diag[:], ones_col[:],
                                     start=False, stop=True)
                # -- early epilog: free U_T/den for the next block right away
                st = dict(b=b, agg=agg)
                rden = sb.tile([128, 1], dt.float32, tag="rden")
                if lay == 1:
                    U_sb = sb.tile([128, 128], dt.bfloat16, tag="usb")
                    nc.vector.tensor_copy(U_sb[:], U_T[:])
                    nc.vector.reciprocal(rden[:], den[:, 0:1])
                    st["U_sb"] = U_sb
                    if DEBUG:
                        nc.sync.dma_start(dbg["usb"][b], U_sb[:])
                        nc.sync.dma_start(dbg["rden"][b], rden[:])
                else:
                    nc.vector.reciprocal(rden[:], agg[:, H2:H2 + 1])
                st["rden"] = rden
                # -- delayed finalize stages
                if pend_B is not None:
                    fin_B(pend_B)
                pend_B = None
                if pend_A is not None:
                    fin_A(pend_A)
                    pend_B = pend_A
                pend_A = st
            if pend_B is not None:
                fin_B(pend_B)
            if pend_A is not None:
                fin_A(pend_A)
                fin_B(pend_A)

        # layer-1 edge phase
        with ExitStack() as ctx1:
            ps_s4 = ctx1.enter_context(tc.tile_pool(name="ps_s4", bufs=2, space="PSUM"))
            ps_tr = ctx1.enter_context(tc.tile_pool(name="ps_tr", bufs=1, space="PSUM"))
            ps_U = ctx1.enter_context(tc.tile_pool(name="ps_U", bufs=2, space="PSUM"))
            ps_den = ctx1.enter_context(tc.tile_pool(name="ps_den", bufs=1, space="PSUM"))
            ps_misc = ctx1.enter_context(tc.tile_pool(name="ps_misc", bufs=2, space="PSUM"))
            edge_phase(1, ps_s4, ps_tr, ps_U, ps_den, ps_misc)

        nc.gpsimd.collective_compute(
            "AllGather", mybir.AluOpType.bypass,
            replica_groups=[list(range(NCORE))],
            ins=[ag2_in[:]], outs=[tbl2[:]])

        # ---------------- layer-2 edge phase + pooling ------------------
        pool_pp = ctx.enter_context(tc.tile_pool(name="poolps", bufs=1, space="PSUM"))
        pool_ps = pool_pp.tile([G, H2 + 4], dt.float32, tag="pool")
        pt_pool = ctx.enter_context(tc.tile_pool(name="ptsb", bufs=1))
        PT_sb = []
        for b in range(NBK):
            t = pt_pool.tile([128, G], dt.bfloat16, tag=f"pt{b}")
            nc.sync.dma_start(t[:], I["PT"][b])
            PT_sb.append(t)
        with ExitStack() as ctx2:
            ps_s4 = ctx2.enter_context(tc.tile_pool(name="ps_s4b", bufs=2, space="PSUM"))
            ps_tr = ctx2.enter_context(tc.tile_pool(name="ps_trb", bufs=1, space="PSUM"))
            ps_U = ctx2.enter_context(tc.tile_pool(name="ps_Ub", bufs=2, space="PSUM"))
            ps_den = ctx2.enter_context(tc.tile_pool(name="ps_denb", bufs=1, space="PSUM"))
            ps_misc = ctx2.enter_context(tc.tile_pool(name="ps_miscb", bufs=2, space="PSUM"))
            edge_phase(2, ps_s4, ps_tr, ps_U, ps_den, ps_misc,
                       pool_ps=pool_ps, PT_sb=PT_sb)

        # ---------------- head -----------------------------------------
        with tc.tile_pool(name="hsb", bufs=2) as hsb, \
             tc.tile_pool(name="hps", bufs=2, space="PSUM") as hps:
            psb = hsb.tile([G, H2], dt.float32, tag="poolsb")
            nc.scalar.copy(psb[:], pool_ps[:, 0:H2])
            nc.sync.dma_start(pool_in[:], psb[:])
            nc.gpsimd.collective_compute(
                "AllReduce", mybir.AluOpType.add,
                replica_groups=[list(range(NCORE))],
                ins=[pool_in[:]], outs=[pool_out[:]])
            pooled = hsb.tile([G, H2], dt.float32, tag="pooled")
            nc.sync.dma_start(pooled[:], pool_out[:])
            pooled_T_ps = hps.tile([H2, G], dt.float32, tag="pooledT")
            nc.tensor.transpose(pooled_T_ps[:], pooled[:], IDENT32[0:G, 0:G])
            pooled_T = hsb.tile([H2, G], dt.float32, tag="pooledTsb")
            nc.scalar.copy(pooled_T[:], pooled_T_ps[:])
            Wd1sb = hsb.tile([H2, HD], dt.float32, tag="wd1")
            nc.sync.dma_start(Wd1sb[:], I["Wd1u"][:])
            h1ps = hps.tile([HD, G], dt.float32, tag="h1")
            nc.tensor.matmul(h1ps[:], Wd1sb[:], pooled_T[:], start=True, stop=True)
            hscale = hsb.tile([HD, 1], dt.float32, tag="hscale")
            nc.sync.dma_start(hscale[:], I["head_scale"][:])
            hbias = hsb.tile([HD, 1], dt.float32, tag="hbias")
            nc.sync.dma_start(hbias[:], I["head_bias"][:])
            th = hsb.tile([HD, G], dt.float32, tag="th")
            nc.scalar.activation(th[:], h1ps[:], AF.Prelu, bias=hbias[:],
                                 scale=hscale[:], alpha=0.1)
            Wd2sb = hsb.tile([HD, OUT], dt.float32, tag="wd2")
            nc.sync.dma_start(Wd2sb[:], I["Wd2"][:])
            ops = hps.tile([OUT, G], dt.float32, tag="ops")
            nc.tensor.matmul(ops[:], Wd2sb[:], th[:], start=True, stop=True)
            bd2sb = hsb.tile([OUT, 1], dt.float32, tag="bd2sb")
            nc.sync.dma_start(bd2sb[:], I["bd2"][:])
            osb = hsb.tile([OUT, G], dt.float32, tag="osb")
            nc.vector.tensor_scalar(osb[:], ops[:], bd2sb[:], None, op0=Alu.add)
            nc.sync.dma_start(out_t[:], osb[:])


def _kernel(inputs, cfg, runner=None, trace=False):
    com, percore, meta = host_prep(inputs, cfg)
    nc = build_program(meta, com, percore[0])
    in_maps = [dict(com, **pc) for pc in percore]
    if runner is None:
        from concourse.bass_utils import run_bass_kernel_spmd
        res = run_bass_kernel_spmd(nc, in_maps, list(range(cfg["NC"])), trace=trace)
        out = np.asarray(res.results[0]["out"])
        return out.T.copy().astype(np.float32), res
    return runner(nc, in_maps)


def kernel(**inputs):
    out, _ = _kernel(inputs, DEFAULT_CFG)
    return out


# revision 39
# speedup vs baseline: 1.2706x; 1.0067x over previous
"""GATv2 x2 + global-mean-pool + MLP head on 8 NeuronCores (Bass/Tile).

Sharding: destination-partitioned. Core c owns nodes [c*NPC, (c+1)*NPC);
it processes every edge whose dst is in its range, so attention softmax
segments are core-local.

Layer 1 gathers RAW node_attr rows (256B) per edge — no xl1 table is
ever materialized.  The per-chunk score xl-term is a matmul of the
transposed gathered rows with Wl1; the aggregation accumulates
U_T[DIN, d] += NA_g.T @ MwT per chunk and applies Wl1 once per block.
Layer 2 AllGathers the raw xl2 table (x1 @ Wl2, no bias) and gathers
its 256B rows per edge.

Host precomputes: loop_attr (self-loop edge features), per-chunk dst
one-hot matrices M [dstrow, edge] (streamed, fp8), dst-local-row
columns, and folds |att| into the weights (channels permuted so
positive-att channels come first; see baseline notes).  All biases are
folded: the score-side bias (bl+br) rides on xr; the output-side bias
(bl+b) is added at block finalize (valid since softmax weights sum to
1).  exp is applied without max-subtraction: logits are O(1) here.
"""

import sys
import numpy as np
import ml_dtypes

sys.path.insert(0, "/opt/trn_rl_repo")

BF16 = ml_dtypes.bfloat16
F8 = ml_dtypes.float8_e4m3

DEFAULT_CFG = dict(
    N=50000, E=500000, G=64,
    DIN=128, ED=32, H1=256, H2=128, HD=64, OUT=8,
    NC=8, HALF=32768,
)


def _roundup(x, m):
    return (x + m - 1) // m * m


def _wrap16(idx, L):
    out = np.full((128, max(L // 16, 1)), -1, np.int16)
    n = len(idx)
    if n:
        pos = np.arange(n)
        out[pos % 16, pos // 16] = idx.astype(np.int16)
    for g in range(1, 8):
        out[g * 16:(g + 1) * 16] = out[0:16]
    return out


def host_prep(inputs, cfg):
    c = dict(cfg)
    N, E, G = c["N"], c["E"], c["G"]
    DIN, ED, H1, H2 = c["DIN"], c["ED"], c["H1"], c["H2"]
    NCORE, HALF = c["NC"], c["HALF"]
    NPC = N // NCORE
    NBK = _roundup(NPC, 128) // 128
    BPC = NBK * 128
    NPAD1 = _roundup(N, 512)
    NPAD2 = NCORE * BPC

    f64 = lambda x: np.asarray(x, np.float64)
    att1, att2 = f64(inputs["att1"]), f64(inputs["att2"])
    a1 = np.maximum(np.abs(att1), 1e-12); s1 = np.where(att1 >= 0, 1.0, -1.0)
    a2 = np.maximum(np.abs(att2), 1e-12); s2 = np.where(att2 >= 0, 1.0, -1.0)
    perm1 = np.argsort(-s1, kind="stable"); P1 = int((s1 > 0).sum())
    perm2 = np.argsort(-s2, kind="stable"); P2 = int((s2 > 0).sum())
    a1p, a2p = a1[perm1], a2[perm2]

    Wl1p = (f64(inputs["Wl1"]) * a1)[:, perm1]
    Wr1p = (f64(inputs["Wr1"]) * a1)[:, perm1]
    We1p = (f64(inputs["We1"]) * a1)[:, perm1]
    bl1p = (f64(inputs["bl1"]) * a1)[perm1]
    br1p = (f64(inputs["br1"]) * a1)[perm1]
    b1p = (f64(inputs["b1"]) * a1)[perm1]

    Wl2u = f64(inputs["Wl2"])[perm1, :] / a1p[:, None]
    Wr2u = f64(inputs["Wr2"])[perm1, :] / a1p[:, None]
    Wl2pp = (Wl2u * a2)[:, perm2]
    Wr2pp = (Wr2u * a2)[:, perm2]
    We2p = (f64(inputs["We2"]) * a2)[:, perm2]
    bl2p = (f64(inputs["bl2"]) * a2)[perm2]
    br2p = (f64(inputs["br2"]) * a2)[perm2]
    b2p = (f64(inputs["b2"]) * a2)[perm2]

    Wd1u = f64(inputs["Wd1"])[perm2, :] / a2p[:, None]
    bs = f64(inputs["bn_gamma"]) / np.sqrt(f64(inputs["bn_var"]) + 1e-5)
    head_scale = bs
    head_bias = (f64(inputs["bd1"]) * bs + f64(inputs["bn_beta"])
                 - f64(inputs["bn_mean"]) * bs)

    src = np.asarray(inputs["edge_src"], np.int64)
    dst = np.asarray(inputs["edge_dst"], np.int64)
    batch = np.asarray(inputs["batch"], np.int64)
    eattr = np.asarray(inputs["edge_attr"], np.float64)

    # loop_attr (self-loop edge features) on host: segment mean of eattr by dst
    deg = np.bincount(dst, minlength=N).astype(np.float64)
    order_d = np.argsort(dst, kind="stable")
    eattr_sorted = eattr[order_d]
    cuts = np.searchsorted(dst[order_d], np.arange(N))
    la = np.zeros((N, ED), np.float64)
    nz = deg > 0
    sums = np.add.reduceat(eattr_sorted, np.minimum(cuts, len(dst) - 1), axis=0)
    la[nz] = sums[nz] / deg[nz][:, None]

    core_of = dst // NPC
    blk_of = (dst % NPC) // 128
    dloc_of = (dst % NPC) % 128

    def layer_streams(row):
        half = (row >= HALF).astype(np.int64)
        cnt = np.zeros((NCORE, NBK, 2), np.int64)
        np.add.at(cnt, (core_of, blk_of, half), 1)
        nmax = cnt.max(axis=0)                         # [NBK, 2] real rows
        nmax[:, 0] = np.maximum(nmax[:, 0], 1)
        seg = _roundup(nmax, 128)                      # [NBK, 2]
        seg[:, 0] = np.maximum(seg[:, 0], 128)
        offs = np.zeros((NBK, 2), np.int64)
        L = 0
        for b in range(NBK):
            for h in range(2):
                offs[b, h] = L
                L += seg[b, h]
        C = L // 128
        key = core_of * (NBK * 2) + blk_of * 2 + half
        order = np.argsort(key, kind="stable")
        ks = key[order]
        idxs = np.zeros((NCORE, 128, L // 16), np.int16)
        eT = np.zeros((NCORE, ED, L), BF16)
        M8 = np.zeros((NCORE, 128, L), F8)
        MT = np.zeros((NCORE, 128, L), BF16)
        bounds = np.searchsorted(ks, np.arange(NCORE * NBK * 2 + 1))
        for cr in range(NCORE):
            for b in range(NBK):
                for h in range(2):
                    k = cr * (NBK * 2) + b * 2 + h
                    m = order[bounds[k]:bounds[k + 1]]
                    n = len(m)
                    o = int(offs[b, h]); sl = int(seg[b, h])
                    if sl == 0:
                        continue
                    nm = int(nmax[b, h])
                    loc_idx = np.full(sl, -1, np.int64)
                    loc_idx[:nm] = 0
                    loc_idx[:n] = row[m] - h * HALF
                    idxs[cr][:, o // 16:(o + sl) // 16] = _wrap16(loc_idx, sl)
                    if n:
                        eT[cr][:, o:o + n] = eattr[m].T.astype(BF16)
                        M8[cr][dloc_of[m], o + np.arange(n)] = F8(1.0)
                        p = np.arange(n)
                        # MT chunk c is the [e, d] one-hot: row e%128 of chunk
                        # (o+e)//128 has a 1 at free-col dloc
                        MT[cr][p % 128, (o + p) // 128 * 128 + dloc_of[m]] = BF16(1.0)
        return dict(seg=seg, offs=offs, nmax=nmax, L=L, C=C, idxs=idxs, eT=eT,
                    M8=M8, MT=MT)

    row1 = src
    row2 = BPC * (src // NPC) + (src % NPC)
    L1s = layer_streams(row1)
    L2s = layer_streams(row2)
    CHMX = int(max(L1s["seg"].max(), L2s["seg"].max())) // 128

    cnts = np.maximum(np.bincount(batch, minlength=G).astype(np.float64), 1.0)
    PT = np.zeros((NCORE, NBK, 128, G), BF16)
    for cr in range(NCORE):
        for b in range(NBK):
            base = cr * NPC + b * 128
            nn = min(128, NPC - b * 128)
            if nn <= 0:
                continue
            gids = batch[base:base + nn]
            PT[cr, b, np.arange(nn), gids] = (1.0 / cnts[gids]).astype(BF16)

    IDENT = np.eye(128, dtype=BF16)
    IDENT32 = np.eye(128, dtype=np.float32)
    ones_col = np.ones((128, 1), BF16)

    na_bf = np.zeros((NPAD1, DIN), BF16)
    na_f32 = np.asarray(inputs["node_attr"], np.float32)
    na_bf[:N] = na_f32.astype(BF16)

    # per-core own-node views (self-loop chunks + xr transform)
    natT = np.zeros((NCORE, DIN, NBK, 128), BF16)
    na_own = np.zeros((NCORE, 128, NBK, DIN), BF16)
    laT = np.zeros((NCORE, ED, NBK, 128), BF16)
    for cr in range(NCORE):
        for b in range(NBK):
            base = cr * NPC + b * 128
            nn = min(128, N - base) if base < N else 0
            nn = min(nn, NPC - b * 128)
            if nn <= 0:
                continue
            natT[cr, :, b, :nn] = na_f32[base:base + nn].T.astype(BF16)
            na_own[cr, :nn, b, :] = na_f32[base:base + nn].astype(BF16)
            laT[cr, :, b, :nn] = la[base:base + nn].T.astype(BF16)

    bcast = lambda v: np.tile(np.asarray(v, np.float32)[None, :], (128, 1)).copy()

    com = dict(
        na_bf=na_bf,
        Wl1p=Wl1p.astype(BF16), Wr1p=Wr1p.astype(BF16), We1p=We1p.astype(BF16),
        Wl2pp=Wl2pp.reshape(H1 // 128, 128, H2).transpose(1, 0, 2).reshape(128, -1).astype(BF16),
        Wr2pp=Wr2pp.reshape(H1 // 128, 128, H2).transpose(1, 0, 2).reshape(128, -1).astype(BF16),
        We2p=We2p.astype(BF16),
        brB1=bcast(bl1p + br1p), bB1=bcast(bl1p + b1p),
        brB2=bcast(bl2p + br2p), bB2=bcast(bl2p + b2p),
        Wd1u=Wd1u.astype(np.float32),
        head_scale=head_scale.astype(np.float32).reshape(-1, 1),
        head_bias=head_bias.astype(np.float32).reshape(-1, 1),
        Wd2=np.asarray(inputs["Wd2"], np.float32),
        bd2=np.asarray(inputs["bd2"], np.float32).reshape(-1, 1),
        IDENT=IDENT, IDENT32=IDENT32,
        ones_col=ones_col,
    )
    percore = []
    for cr in range(NCORE):
        percore.append(dict(
            idxs1=L1s["idxs"][cr], eT1=L1s["eT"][cr], M81=L1s["M8"][cr],
            MT1=L1s["MT"][cr],
            idxs2=L2s["idxs"][cr], eT2=L2s["eT"][cr], M82=L2s["M8"][cr],
            MT2=L2s["MT"][cr],
            PT=PT[cr], natT=natT[cr], na_own=na_own[cr], laT=laT[cr],
        ))
    meta = dict(cfg=c, NPC=NPC, NBK=NBK, BPC=BPC, NPAD1=NPAD1, NPAD2=NPAD2,
                P1=P1, P2=P2, L1=L1s, L2=L2s, CHMX=CHMX)
    return com, percore, meta


def build_program(meta, com, pc0):
    import concourse.bass as bass
    import concourse.tile as tile
    from concourse import bacc, mybir
    from concourse import library_config

    c = meta["cfg"]
    G, H2, OUT = c["G"], c["H2"], c["OUT"]
    NCORE = c["NC"]
    BPC = meta["BPC"]
    NPAD2 = meta["NPAD2"]
    dt = mybir.dt

    nc = bacc.Bacc("TRN2", target_bir_lowering=False, debug=False,
                   num_devices=NCORE)

    dmap = {np.dtype(np.float32): dt.float32, np.dtype(BF16): dt.bfloat16,
            np.dtype(np.int16): dt.int16, np.dtype(F8): dt.float8e4}
    I = {}
    for d in (com, pc0):
        for k, a in d.items():
            I[k] = nc.dram_tensor(k, list(a.shape), dmap[a.dtype],
                                  kind="ExternalInput")

    out_t = nc.dram_tensor("out", [OUT, G], dt.float32, kind="ExternalOutput")
    NBK = meta["NBK"]
    H1 = c["H1"]
    dbg = dict(
        rden=nc.dram_tensor("dbg_rden", [NBK, 128, 1], dt.float32),
        es=nc.dram_tensor("dbg_es", [NBK, 128, 1], dt.float32),
        usb=nc.dram_tensor("dbg_usb", [NBK, 128, 128], dt.bfloat16),
        t2=nc.dram_tensor("dbg_t2", [NBK, 128, H1], dt.float32),
        e4=nc.dram_tensor("dbg_e4", [NBK, 128, 2], dt.float32),
    )
    ag2_in = nc.dram_tensor("ag2_in", [BPC, H2], dt.bfloat16)
    tbl2 = nc.dram_tensor("tbl2", [NPAD2, H2], dt.bfloat16, addr_space="Shared")
    pool_in = nc.dram_tensor("pool_in", [G, H2], dt.float32)
    pool_out = nc.dram_tensor("pool_out", [G, H2], dt.float32, addr_space="Shared")

    with tile.TileContext(nc) as tc:
        _body(nc, tc, I, out_t, ag2_in, tbl2, pool_in, pool_out,
              meta, bass, tile, mybir, library_config, dbg=dbg)
    nc.compile()
    return nc


DEBUG = False


def _body(nc, tc, I, out_t, ag2_in, tbl2, pool_in, pool_out,
          meta, bass, tile, mybir, library_config, dbg=None):
    from contextlib import ExitStack

    c = meta["cfg"]
    G = c["G"]
    DIN, ED, H1, H2, HD, OUT = c["DIN"], c["ED"], c["H1"], c["H2"], c["HD"], c["OUT"]
    NCORE, HALF = c["NC"], c["HALF"]
    NPC, NBK, BPC = meta["NPC"], meta["NBK"], meta["BPC"]
    NPAD1, NPAD2 = meta["NPAD1"], meta["NPAD2"]
    P1, P2 = meta["P1"], meta["P2"]
    CHMX = meta["CHMX"]
    AF = mybir.ActivationFunctionType
    dt = mybir.dt
    Alu = mybir.AluOpType
    ds = bass.ds

    nc.gpsimd.load_library(library_config.mlp)
    pid = nc.partition_id()

    ctx = ExitStack()
    with ctx:
        consts = ctx.enter_context(tc.tile_pool(name="consts", bufs=1))

        def cload(name):
            a = I[name]
            t = consts.tile(list(a.shape), a.dtype, tag=name)
            nc.sync.dma_start(t[:], a[:])
            return t

        IDENT = cload("IDENT")
        IDENT32 = cload("IDENT32")
        ones_col = cload("ones_col")
        Wl1p = cload("Wl1p"); Wr1p = cload("Wr1p"); We1p = cload("We1p")
        Wl2pp = cload("Wl2pp"); Wr2pp = cload("Wr2pp"); We2p = cload("We2p")
        brB1 = cload("brB1"); bB1 = cload("bB1")
        brB2 = cload("brB2"); bB2 = cload("bB2")
        natT = cload("natT"); na_own = cload("na_own"); laT = cload("laT")

        res = ctx.enter_context(tc.tile_pool(name="res", bufs=1))
        xr1_nm = res.tile([128, NBK, H1], dt.bfloat16, tag="xr1")
        x1_T = res.tile([128, H1 // 128, BPC], dt.bfloat16, tag="x1T")
        xr2_nm = res.tile([128, NBK, H2], dt.bfloat16, tag="xr2")

        # ---------------- phase 0: xr1 for own nodes -------------------
        with tc.tile_pool(name="p0ps", bufs=2, space="PSUM") as p0ps:
            for b in range(NBK):
                ps = p0ps.tile([128, H1], dt.float32, tag="xr1ps")
                nc.tensor.matmul(ps[:], natT[:, b, :], Wr1p[:],
                                 start=True, stop=True)
                nc.vector.tensor_tensor(xr1_nm[:, b, :], ps[:], brB1[:], op=Alu.add)

        # ---------------- shared pools for both edge phases ------------
        sb = ctx.enter_context(tc.tile_pool(name="sb", bufs=4))
        sbg = ctx.enter_context(tc.tile_pool(name="sbg", bufs=6))
        nag = ctx.enter_context(tc.tile_pool(name="nag", bufs=CHMX + 4))
        pre = ctx.enter_context(tc.tile_pool(name="pre", bufs=1))



        # ---------------- shared edge phase ----------------------------
        def edge_phase(lay, ps_s4, ps_tr, ps_U, ps_den, ps_misc,
                       pool_ps=None, PT_sb=None):
            H = H1 if lay == 1 else H2
            Ppos = P1 if lay == 1 else P2
            We = We1p if lay == 1 else We2p
            xr_nm = xr1_nm if lay == 1 else xr2_nm
            bB = bB1 if lay == 1 else bB2
            sfx = str(lay)
            Ls = meta["L" + sfx]
            seg, offs, nmax = Ls["seg"], Ls["offs"], Ls["nmax"]
            L = int(Ls["L"]); C = int(Ls["C"])
            if lay == 1:
                tlo = I["na_bf"][0:HALF, :]
                thi = I["na_bf"][HALF:NPAD1, :]
            else:
                tlo = tbl2[0:HALF, :]
                thi = tbl2[HALF:NPAD2, :]

            idx_all = pre.tile([128, L // 16], dt.int16, tag="idx" + sfx)
            nc.sync.dma_start(idx_all[:], I["idxs" + sfx][:])

            # --- two-stage software-pipelined block loop: finalize stage A
            # (normalize + relu) runs one block behind the edge work, stage B
            # (transposes + layer-2 transforms / pooling) two blocks behind,
            # so the long cross-engine chains never stall the next block's
            # tensor stream.
            def fin_A(st):
                b = st["b"]
                if lay == 1:
                    aggf = ps_misc.tile([128, H1], dt.float32, tag="misc")
                    nc.tensor.matmul(aggf[:], st["U_sb"][:], Wl1p[:],
                                     start=True, stop=True)
                    aggp = aggf
                else:
                    aggp = st["agg"]
                t1 = sb.tile([128, H], dt.float32, tag="t1")
                nc.scalar.activation(t1[:], aggp[:, 0:H], AF.Copy,
                                     scale=st["rden"][:])
                t2 = sb.tile([128, H], dt.float32, tag="t2")
                nc.vector.tensor_tensor(t2[:], t1[:], bB[:], op=Alu.add)
                if DEBUG and lay == 1:
                    nc.sync.dma_start(dbg["t2"][b], t2[:])
                x_nm = sb.tile([128, H], dt.bfloat16, tag="xnm")
                nc.scalar.activation(x_nm[:], t2[:], AF.Relu)
                st["x_nm"] = x_nm

            def fin_B(st):
                b = st["b"]
                x_nm = st["x_nm"]
                if lay == 1:
                    for hh in range(H1 // 128):
                        tp = ps_tr.tile([128, 128], dt.bfloat16, tag="tr")
                        nc.tensor.transpose(tp[:], x_nm[:, hh * 128:(hh + 1) * 128],
                                            IDENT[:])
                        nc.scalar.copy(x1_T[:, hh, b * 128:(b + 1) * 128], tp[:])
                    psl = ps_misc.tile([128, H2], dt.float32, tag="misc")
                    for hh in range(H1 // 128):
                        nc.tensor.matmul(psl[:], x1_T[:, hh, b * 128:(b + 1) * 128],
                                         Wl2pp[:, hh * H2:(hh + 1) * H2],
                                         start=(hh == 0), stop=(hh == H1 // 128 - 1))
                    sbx = sb.tile([128, H2], dt.bfloat16, tag="sbx")
                    nc.vector.tensor_copy(sbx[:], psl[:])
                    nc.sync.dma_start(ag2_in[b * 128:(b + 1) * 128, :], sbx[:])
                    psr = ps_misc.tile([128, H2], dt.float32, tag="misc")
                    for hh in range(H1 // 128):
                        nc.tensor.matmul(psr[:], x1_T[:, hh, b * 128:(b + 1) * 128],
                                         Wr2pp[:, hh * H2:(hh + 1) * H2],
                                         start=(hh == 0), stop=(hh == H1 // 128 - 1))
                    nc.vector.tensor_tensor(xr2_nm[:, b, :], psr[:], brB2[:],
                                            op=Alu.add)
                else:
                    nc.tensor.matmul(pool_ps[:, 0:H2], PT_sb[b][:], x_nm[:],
                                     start=(b == 0), stop=(b == NBK - 1))

            pend_A = None
            pend_B = None
            for b in range(NBK):
                if lay == 1:
                    U_T = ps_U.tile([128, 128], dt.float32, tag="UT")
                    den = ps_den.tile([128, 8], dt.float32, tag="den")
                    agg = None
                else:
                    agg = ps_U.tile([128, H2 + 8], dt.float32, tag="agg2")
                    xlw = sbg.tile([128, H2], dt.bfloat16, tag="xlw")
                    nc.sync.dma_start(xlw[:], tbl2[ds(pid * BPC + b * 128, 128), :])
                # -- self-loop scores up-front (consts only; diag is ready
                # long before the block-tail aggregation needs it)
                s_s = ps_s4.tile([128, 2, H], dt.float32, tag="s4")
                if lay == 1:
                    nc.tensor.matmul(s_s[:, 0, :], natT[:, b, :], Wl1p[:],
                                     start=True, stop=False)
                else:
                    nc.tensor.matmul(s_s[:, 0, :], IDENT[:], xlw[:],
                                     start=True, stop=False)
                nc.tensor.matmul(s_s[:, 0, :], IDENT[:], xr_nm[:, b, :],
                                 start=False, stop=False)
                nc.tensor.matmul(s_s[:, 0, :], laT[:, b, :], We[:],
                                 start=False, stop=True)
                ls_s = sb.tile([128, 2, H], dt.bfloat16, tag="ls4")
                if Ppos > 0:
                    nc.scalar.activation(ls_s[:, 0, 0:Ppos], s_s[:, 0, 0:Ppos],
                                         AF.Prelu, alpha=0.2)
                if Ppos < H:
                    nc.scalar.activation(ls_s[:, 0, Ppos:H], s_s[:, 0, Ppos:H],
                                         AF.Prelu, scale=-0.2, alpha=5.0)
                es = sb.tile([128, 1], dt.float32, tag="es")
                nc.vector.reduce_sum(es[:], ls_s[:, 0:1, :], axis=mybir.AxisListType.X)
                ws = sb.tile([128, 1], dt.float32, tag="ws")
                nc.scalar.activation(ws[:], es[:], AF.Exp)
                if DEBUG and lay == 1:
                    nc.sync.dma_start(dbg["es"][b], es[:])
                diag = sb.tile([128, 128], dt.bfloat16, tag="diag")
                nc.scalar.activation(diag[:], IDENT[:], AF.Copy, scale=ws[:])

                first = True
                pend_g = None   # one-group-delayed MwT build + aggregation

                def flush_group():
                    nonlocal first, pend_g
                    if pend_g is None:
                        return
                    xlg_, po_, nch_, MTs_, w4_ = pend_g
                    for j in range(nch_):
                        cs = xlg_[:, po_ // 128 + j, :]
                        ec = slice(po_ + j * 128, po_ + (j + 1) * 128)
                        MwT = sb.tile([128, 128], dt.bfloat16, tag="mwt")
                        nc.scalar.activation(MwT[:], MTs_[:, ec], AF.Copy,
                                             scale=w4_[:, j:j + 1])
                        if lay == 1:
                            nc.tensor.matmul(U_T[:], cs, MwT[:],
                                             start=first, stop=False)
                            nc.tensor.matmul(den[:, 0:1], MwT[:], ones_col[:],
                                             start=first, stop=False)
                        else:
                            nc.tensor.matmul(agg[:, 0:H2], MwT[:], cs,
                                             start=first, stop=False)
                            nc.tensor.matmul(agg[:, H2:H2 + 1], MwT[:], ones_col[:],
                                             start=False, stop=False)
                        first = False
                    pend_g = None

                for h in range(2):
                    sl = int(seg[b, h]); o = int(offs[b, h])
                    if sl == 0:
                        continue
                    nch_all = sl // 128
                    xlg = sbg.tile([128, nch_all, 128], dt.bfloat16, tag="xlg")
                    nm = int(nmax[b, h])
                    if nm < sl:
                        # slots [nm:sl) are skipped by the gather (trailing
                        # negative idxs) and would hold stale SBUF data; zero
                        # the last chunk first so downstream exp/matmuls see
                        # finite values (the gather overwrites real rows).
                        nc.vector.memset(xlg[:, nch_all - 1, :], 0.0)
                    nc.gpsimd.dma_gather(xlg[:], thi if h else tlo,
                                         idx_all[:, o // 16:(o + sl) // 16],
                                         sl, nm, 128)
                    eTs = sb.tile([32, CHMX * 128], dt.bfloat16, tag="eT")
                    nc.sync.dma_start(eTs[:, :sl], I["eT" + sfx][:, o:o + sl])
                    M8s = sb.tile([128, CHMX * 128], dt.float8e4, tag="M8")
                    nc.scalar.dma_start(M8s[:, :sl], I["M8" + sfx][:, o:o + sl])
                    MTs = sb.tile([128, CHMX * 128], dt.bfloat16, tag="MT")
                    nc.scalar.dma_start(MTs[:, :sl], I["MT" + sfx][:, o:o + sl])
                    nagTs = []
                    if lay == 1:
                        # transpose all chunks up-front so the per-group score
                        # matmuls never wait on a fresh transpose+copy pair
                        for j in range(nch_all):
                            tp = ps_tr.tile([128, 128], dt.bfloat16, tag="tr")
                            nc.tensor.transpose(tp[:], xlg[:, j, :], IDENT[:])
                            nagT = nag.tile([128, 128], dt.bfloat16, tag="nagT")
                            nc.vector.tensor_copy(nagT[:], tp[:])
                            nagTs.append(nagT)
                    for po in range(0, sl, 256):
                        pl = min(256, sl - po)
                        nch = pl // 128
                        jj0 = (o + po) // 128
                        s4 = ps_s4.tile([128, 2, H], dt.float32, tag="s4")
                        for j in range(nch):
                            cs = xlg[:, po // 128 + j, :]
                            ec = slice(po + j * 128, po + (j + 1) * 128)
                            if lay == 1:
                                nc.tensor.matmul(s4[:, j, :], nagTs[po // 128 + j][:],
                                                 Wl1p[:], start=(j == 0), stop=False)
                            else:
                                nc.tensor.matmul(s4[:, j, :], IDENT[:], cs,
                                                 start=(j == 0), stop=False)
                            nc.tensor.matmul(s4[:, j, :], M8s[:, ec], xr_nm[:, b, :],
                                             start=False, stop=False)
                            nc.tensor.matmul(s4[:, j, :], eTs[:, ec], We[:],
                                             start=False, stop=(j == nch - 1))
                        ls4 = sb.tile([128, 2, H], dt.bfloat16, tag="ls4")
                        if Ppos > 0:
                            nc.scalar.activation(ls4[:, :nch, 0:Ppos], s4[:, :nch, 0:Ppos],
                                                 AF.Prelu, alpha=0.2)
                        if Ppos < H:
                            nc.scalar.activation(ls4[:, :nch, Ppos:H], s4[:, :nch, Ppos:H],
                                                 AF.Prelu, scale=-0.2, alpha=5.0)
                        e4 = sb.tile([128, 2], dt.float32, tag="e4")
                        nc.vector.reduce_sum(e4[:, :nch], ls4[:, :nch, :],
                                             axis=mybir.AxisListType.X)
                        w4 = sb.tile([128, 2], dt.float32, tag="w4")
                        nc.scalar.activation(w4[:, :nch], e4[:, :nch], AF.Exp)
                        if DEBUG and lay == 1 and h == 0 and po == 0:
                            nc.sync.dma_start(dbg["e4"][b], e4[:])
                        flush_group()
                        pend_g = (xlg, po, nch, MTs, w4)
                flush_group()
                # -- self-loop aggregation (diag ready since block start)
                if lay == 1:
                    nc.tensor.matmul(U_T[:], na_own[:, b, :], diag[:],
                                     start=False, stop=True)
                    nc.tensor.matmul(den[:, 0:1], diag[:], ones_col[:],
                                     start=False, stop=True)
                else:
                    nc.tensor.matmul(agg[:, 0:H2], diag[:], xlw[:],
                                     start=False, stop=False)
                    nc.tensor.matmul(agg[:, H2:H2 + 1], diag[:], ones_col[:],
                                     start=False, stop=True)
                # -- early epilog: free U_T/den for the next block right away
                st = dict(b=b, agg=agg)
                rden = sb.tile([128, 1], dt.float32, tag="rden")
                if lay == 1:
                    U_sb = sb.tile([128, 128], dt.bfloat16, tag="usb")
                    nc.vector.tensor_copy(U_sb[:], U_T[:])
                    nc.vector.reciprocal(rden[:], den[:, 0:1])
                    st["U_sb"] = U_sb
                    if DEBUG:
                        nc.sync.dma_start(dbg["usb"][b], U_sb[:])
                        nc.sync.dma_start(dbg["rden"][b], rden[:])
                else:
                    nc.vector.reciprocal(rden[:], agg[:, H2:H2 + 1])
                st["rden"] = rden
                # -- delayed finalize stages
                if pend_B is not None:
                    fin_B(pend_B)
                pend_B = None
                if pend_A is not None:
                    fin_A(pend_A)
                    pend_B = pend_A
                pend_A = st
            if pend_B is not None:
                fin_B(pend_B)
            if pend_A is not None:
                fin_A(pend_A)
                fin_B(pend_A)

        # layer-1 edge phase
        with ExitStack() as ctx1:
            ps_s4 = ctx1.enter_context(tc.tile_pool(name="ps_s4", bufs=2, space="PSUM"))
            ps_tr = ctx1.enter_context(tc.tile_pool(name="ps_tr", bufs=1, space="PSUM"))
            ps_U = ctx1.enter_context(tc.tile_pool(name="ps_U", bufs=2, space="PSUM"))
            ps_den = ctx1.enter_context(tc.tile_pool(name="ps_den", bufs=1, space="PSUM"))
            ps_misc = ctx1.enter_context(tc.tile_pool(name="ps_misc", bufs=2, space="PSUM"))
            edge_phase(1, ps_s4, ps_tr, ps_U, ps_den, ps_misc)

        nc.gpsimd.collective_compute(
            "AllGather", mybir.AluOpType.bypass,
            replica_groups=[list(range(NCORE))],
            ins=[ag2_in[:]], outs=[tbl2[:]])

        # ---------------- layer-2 edge phase + pooling ------------------
        pool_pp = ctx.enter_context(tc.tile_pool(name="poolps", bufs=1, space="PSUM"))
        pool_ps = pool_pp.tile([G, H2 + 4], dt.float32, tag="pool")
        pt_pool = ctx.enter_context(tc.tile_pool(name="ptsb", bufs=1))
        PT_sb = []
        for b in range(NBK):
            t = pt_pool.tile([128, G], dt.bfloat16, tag=f"pt{b}")
            nc.sync.dma_start(t[:], I["PT"][b])
            PT_sb.append(t)
        with ExitStack() as ctx2:
            ps_s4 = ctx2.enter_context(tc.tile_pool(name="ps_s4b", bufs=2, space="PSUM"))
            ps_tr = ctx2.enter_context(tc.tile_pool(name="ps_trb", bufs=1, space="PSUM"))
            ps_U = ctx2.enter_context(tc.tile_pool(name="ps_Ub", bufs=2, space="PSUM"))
            ps_den = ctx2.enter_context(tc.tile_pool(name="ps_denb", bufs=1, space="PSUM"))
            ps_misc = ctx2.enter_context(tc.tile_pool(name="ps_miscb", bufs=2, space="PSUM"))
            edge_phase(2, ps_s4, ps_tr, ps_U, ps_den, ps_misc,
                       pool_ps=pool_ps, PT_sb=PT_sb)

        # ---------------- head -----------------------------------------
        with tc.tile_pool(name="hsb", bufs=2) as hsb, \
             tc.tile_pool(name="hps", bufs=2, space="PSUM") as hps:
            psb = hsb.tile([G, H2], dt.float32, tag="poolsb")
            nc.scalar.copy(psb[:], pool_ps[:, 0:H2])
            nc.sync.dma_start(pool_in[:], psb[:])
            nc.gpsimd.collective_compute(
                "AllReduce", mybir.AluOpType.add,
                replica_groups=[list(range(NCORE))],
                ins=[pool_in[:]], outs=[pool_out[:]])
            pooled = hsb.tile([G, H2], dt.float32, tag="pooled")
            nc.sync.dma_start(pooled[:], pool_out[:])
            pooled_T_ps = hps.tile([H2, G], dt.float32, tag="pooledT")
            nc.tensor.transpose(pooled_T_ps[:], pooled[:], IDENT32[0:G, 0:G])
            pooled_T = hsb.tile([H2, G], dt.float32, tag="pooledTsb")
            nc.scalar.copy(pooled_T[:], pooled_T_ps[:])
            Wd1sb = hsb.tile([H2, HD], dt.float32, tag="wd1")
            nc.sync.dma_start(Wd1sb[:], I["Wd1u"][:])
            h1ps = hps.tile([HD, G], dt.float32, tag="h1")
            nc.tensor.matmul(h1ps[:], Wd1sb[:], pooled_T[:], start=True, stop=True)
            hscale = hsb.tile([HD, 1], dt.float32, tag="hscale")
            nc.sync.dma_start(hscale[:], I["head_scale"][:])
            hbias = hsb.tile([HD, 1], dt.float32, tag="hbias")
            nc.sync.dma_start(hbias[:], I["head_bias"][:])
            th = hsb.tile([HD, G], dt.float32, tag="th")
            nc.scalar.activation(th[:], h1ps[:], AF.Prelu, bias=hbias[:],
                                 scale=hscale[:], alpha=0.1)
            Wd2sb = hsb.tile([HD, OUT], dt.float32, tag="wd2")
            nc.sync.dma_start(Wd2sb[:], I["Wd2"][:])
            ops = hps.tile([OUT, G], dt.float32, tag="ops")
            nc.tensor.matmul(ops[:], Wd2sb[:], th[:], start=True, stop=True)
            bd2sb = hsb.tile([OUT, 1], dt.float32, tag="bd2sb")
            nc.sync.dma_start(bd2sb[:], I["bd2"][:])
            osb = hsb.tile([OUT, G], dt.float32, tag="osb")
            nc.vector.tensor_scalar(osb[:], ops[:], bd2sb[:], None, op0=Alu.add)
            nc.sync.dma_start(out_t[:], osb[:])


def _kernel(inputs, cfg, runner=None, trace=False):
    com, percore, meta = host_prep(inputs, cfg)
    nc = build_program(meta, com, percore[0])
    in_maps = [dict(com, **pc) for pc in percore]
    if runner is None:
        from concourse.bass_utils import run_bass_kernel_spmd
        res = run_bass_kernel_spmd(nc, in_maps, list(range(cfg["NC"])), trace=trace)
        out = np.asarray(res.results[0]["out"])
        return out.T.copy().astype(np.float32), res
    return runner(nc, in_maps)


def kernel(**inputs):
    out, _ = _kernel(inputs, DEFAULT_CFG)
    return out
